# revision 19
# baseline (speedup 1.0000x reference)
"""AttnDecoderRNN step on 8 TRN2 NeuronCores, tensor-parallel.

Strategy (contraction sharding + AllReduce between stages):
  stage1 attn logits : attn_W.T row-sharded (512 emb rows + 512 hid rows
                       per core), partial [512] -> AR -> +bias -> softmax
  stage2 attn_applied: latent_out column-sharded; each core computes its
                       own 512-wide slice (no collective)
  stage3 comb        : comb_W.T row-sharded (512 emb + 512 attn rows per
                       core), partial [4096] -> AR -> slice readback ->
                       +bias -> relu -> x slice
  stage4 GRU gates   : Wih/Whh contraction-sharded (512 x-rows, 256 h-rows
                       per core); r/z gates pre-summed (gi+gh) in PSUM, the
                       n gate halves kept split; partial [16384] -> AR ->
                       +bias -> gate math -> h_new
  stage5 vocab proj  : out_W output-sharded (6656 rows per core, padded to
                       53248); +bias, local sumexp -> AR(scalar) -> logp
Embedding lookup: emb column-sharded; one indirect-DMA row gather per core.
All weights are pre-transposed/tiled on the host so every DMA is contiguous
and every matmul operand sits at partition base 0.  Matvec form:
psum[1,N] += lhsT(x column tile [128,1]).T @ rhs(W.T tile [128,N]).
Biases are applied once, post-AllReduce, as elementwise adds.
"""
import os
import sys

sys.path.insert(0, "/opt/trn_rl_repo")

import numpy as np

from concourse import bass, bacc, tile, mybir
from concourse.bass_utils import run_bass_kernel_spmd

V, H, L, Hh = 50257, 4096, 512, 2048
M = 8               # cores
VC = 6656           # per-core padded vocab rows (13 * 512)
VP = VC * M         # 53248
NEG = -1.0e30

F32 = mybir.dt.float32
F32R = mybir.dt.float32r
BF16 = mybir.dt.bfloat16
I32 = mybir.dt.int32

# 'f32' (exact, PE 4cyc/row) or 'bf16' (half DMA bytes, PE 1cyc/row)
DT_MODE = os.environ.get("BASS_KERNEL_DT", "f32")

AF = mybir.ActivationFunctionType
ALU = mybir.AluOpType
RG = [list(range(M))]

_CACHE = {}


def _mm(nc, out, lhsT, rhs, start, stop, dt_mode):
    if dt_mode == "f32r":
        lhsT = lhsT.bitcast(F32R)
        rhs = rhs.bitcast(F32R)
    nc.tensor.matmul(out, lhsT, rhs, start=start, stop=stop)


def _build(dt_mode):
    wdt = {"f32": F32, "f32r": F32, "bf16": BF16, "mixed": BF16}[dt_mode]
    wdt_at = F32 if dt_mode in ("f32", "f32r", "mixed") else BF16
    pair = dt_mode == "mixed"
    nv = 2 if pair else 1
    nc = bacc.Bacc("TRN2", target_bir_lowering=False, debug=False,
                   enable_asserts=True, num_devices=M)

    def din(name, shape, dt=F32):
        return nc.dram_tensor(name, shape, dt, kind="ExternalInput").ap()

    def dout(name, shape, dt=F32):
        return nc.dram_tensor(name, shape, dt, kind="ExternalOutput").ap()

    embp = din("embp", [V, 512])
    idx2 = din("idx2", [2, 1], I32)
    xrows = din("xrows", [4, 1], I32)
    id32_d = din("id32", [32, 32])
    hkc_d = din("hkc", [128, 4])
    hfkc_d = din("hfkc", [128, 2])
    hbkc_d = din("hbkc", [128, 2])
    hprev_d = din("hprev", [32, 128])
    abcm_d = din("abcm", [4, 128])
    cbcm_d = din("cbcm", [4, 128])
    gbr_d = din("gbr", [32, 128])
    gbz_d = din("gbz", [32, 128])
    gbi_d = din("gbi", [32, 128])
    gbh_d = din("gbh", [32, 128])
    obcm_d = din("obcm", [13, 512])
    a1_d = din("a1", [128, 4, 512], wdt_at)
    a2_d = din("a2", [128, 4, 512], wdt_at)
    lat_d = din("lat", [128, 4, 512], wdt_at)
    c1_d = din("c1", [128, 8, nv, 4, 512], wdt)
    c2_d = din("c2", [128, 8, nv, 4, 512], wdt)
    wrzf_d = din("wrzf", [128, 8, nv, 4, 512], wdt)
    wrzb_d = din("wrzb", [128, 8, nv, 4, 512], wdt)
    whrzf_d = din("whrzf", [128, 8, nv, 2, 512], wdt)
    whrzb_d = din("whrzb", [128, 8, nv, 2, 512], wdt)
    winf_d = din("winf", [128, 4, nv, 4, 512], wdt)
    winb_d = din("winb", [128, 4, nv, 4, 512], wdt)
    whnf_d = din("whnf", [128, 4, nv, 2, 512], wdt)
    whnb_d = din("whnb", [128, 4, nv, 2, 512], wdt)
    wout_d = din("wout", [128, 13, 32, 512], wdt)

    lp_d = dout("lp", [13, 512])
    nh_d = dout("nh", [32, 128])
    aw_d = dout("aw", [4, 128])

    with tile.TileContext(nc) as tc:
        PF = 4 if wdt == BF16 else 1
        with tc.tile_pool(name="const", bufs=1) as cp, \
             tc.tile_pool(name="vec", bufs=1) as vp, \
             tc.tile_pool(name="psv", bufs=2, space="PSUM") as psv, \
             tc.tile_pool(name="s5w", bufs=max(PF, 2)) as s5w, \
             tc.tile_pool(name="dram", bufs=1, space="DRAM") as dr:

            id32 = cp.tile([32, 32], F32)
            nc.sync.dma_start(id32[:], id32_d[:])
            ones4 = cp.tile([4, 1], F32)
            nc.vector.memset(ones4[:], 1.0)
            ones13 = cp.tile([13, 1], F32)
            nc.vector.memset(ones13[:], 1.0)
            ones113 = cp.tile([1, 13], F32)
            nc.vector.memset(ones113[:], 1.0)
            ones14 = cp.tile([1, 4], F32)
            nc.vector.memset(ones14[:], 1.0)

            idx2_s = cp.tile([2, 1], I32)
            nc.sync.dma_start(idx2_s[:], idx2[:])
            xrows_s = cp.tile([4, 1], I32)
            nc.sync.dma_start(xrows_s[:], xrows[:])
            hkc = cp.tile([128, 4], F32)
            nc.sync.dma_start(hkc[:], hkc_d[:])
            hfkc = cp.tile([128, 2], F32)
            nc.sync.dma_start(hfkc[:], hfkc_d[:])
            hbkc = cp.tile([128, 2], F32)
            nc.sync.dma_start(hbkc[:], hbkc_d[:])
            hprev = cp.tile([32, 128], F32)
            nc.sync.dma_start(hprev[:], hprev_d[:])
            abcm = cp.tile([4, 128], F32)
            nc.sync.dma_start(abcm[:], abcm_d[:])
            cbcm = cp.tile([4, 128], F32)
            nc.sync.dma_start(cbcm[:], cbcm_d[:])
            gbr = cp.tile([32, 128], F32)
            nc.sync.dma_start(gbr[:], gbr_d[:])
            gbz = cp.tile([32, 128], F32)
            nc.sync.dma_start(gbz[:], gbz_d[:])
            gbi = cp.tile([32, 128], F32)
            nc.sync.dma_start(gbi[:], gbi_d[:])
            gbh = cp.tile([32, 128], F32)
            nc.sync.dma_start(gbh[:], gbh_d[:])
            obcm = cp.tile([13, 512], F32)
            nc.sync.dma_start(obcm[:], obcm_d[:])

            s1w_cm = tc.tile_pool(name="s1w", bufs=1)
            s1w = s1w_cm.__enter__()
            a1 = s1w.tile([128, 4, 512], wdt_at)
            nc.sync.dma_start(a1[:], a1_d[:])
            a2 = s1w.tile([128, 4, 512], wdt_at)
            nc.sync.dma_start(a2[:], a2_d[:])
            lat = s1w.tile([128, 4, 512], wdt_at)
            nc.sync.dma_start(lat[:], lat_d[:])

            # collective bounce buffers
            cc1_in = dr.tile([1, 512], F32)
            cc1_out = dr.tile([1, 512], F32, addr_space="Shared")
            cc2_in = dr.tile([8, 512], F32)
            cc2_out = dr.tile([8, 512], F32, addr_space="Shared")
            cc3_in = dr.tile([48, 512], F32)
            cc3_out = dr.tile([48, 512], F32, addr_space="Shared")
            cc4_in = dr.tile([1, 1], F32)
            cc4_out = dr.tile([1, 1], F32, addr_space="Shared")

            def row_to_cols(row, n, name):
                """[1, 128*n] SBUF row -> [128, n] col tiles via PE transposes."""
                pc = psv.tile([128, n], F32, name=f"{name}_ps", tag="ptrans")
                for t in range(n):
                    nc.tensor.matmul(pc[:, t:t + 1], row[0:1, bass.ts(t, 128)],
                                     id32[0:1, 0:1], is_transpose=True,
                                     start=(t == 0), stop=(t == n - 1))
                out = vp.tile([128, n], F32, name=f"{name}_sb", tag=f"{name}_sb")
                nc.vector.tensor_copy(out[:], pc[:])
                return out

            def cast_cols(cols, name):
                if wdt != BF16:
                    return cols
                cb = vp.tile(list(cols.shape), BF16, name=f"{name}_bf",
                             tag=f"{name}_bf")
                nc.vector.tensor_copy(cb[:], cols[:])
                return cb

            def split_cols(cols, name):
                """f32 cols -> (hi, lo) bf16 pair; or single bf16/f32 view."""
                if not pair:
                    if wdt == BF16:
                        cb = vp.tile(list(cols.shape), BF16, name=f"{name}_h",
                                     tag=f"{name}_h")
                        nc.vector.tensor_copy(cb[:], cols[:])
                        return (cb,)
                    return (cols,)
                hi = vp.tile(list(cols.shape), BF16, name=f"{name}_h",
                             tag=f"{name}_h")
                nc.vector.tensor_copy(hi[:], cols[:])
                hif = vp.tile(list(cols.shape), F32, name=f"{name}_hf",
                              tag=f"{name}_hf")
                nc.vector.tensor_copy(hif[:], hi[:])
                dif = vp.tile(list(cols.shape), F32, name=f"{name}_d",
                              tag=f"{name}_d")
                nc.vector.tensor_tensor(dif[:], cols[:], hif[:], op=ALU.subtract)
                lo = vp.tile(list(cols.shape), BF16, name=f"{name}_l",
                             tag=f"{name}_l")
                nc.vector.tensor_copy(lo[:], dif[:])
                return (hi, lo)

            def mm_group(ps, lhs_sets, first, last):
                """Accumulate sum_j lhs_sets[j] @ tiles_j into ps.
                lhs_sets: list of (cols_tuple, wtile_ap, nt); wtile_ap indexed
                [v, t] in pair mode else [t]."""
                seq = []
                for cols, wtile, nt in lhs_sets:
                    for t in range(nt):
                        seq.append((cols[0][:, t:t + 1], wtile[:, 0, t, :]))
                        if pair:
                            seq.append((cols[1][:, t:t + 1], wtile[:, 0, t, :]))
                            seq.append((cols[0][:, t:t + 1], wtile[:, 1, t, :]))
                for i, (l, r) in enumerate(seq):
                    nc.tensor.matmul(ps, l, r,
                                     start=(first and i == 0),
                                     stop=(last and i == len(seq) - 1))

            # ---------------- embedding gather ----------------
            ge = vp.tile([2, 512], F32)
            nc.gpsimd.indirect_dma_start(
                out=ge[:], out_offset=None, in_=embp[:],
                in_offset=bass.IndirectOffsetOnAxis(ap=idx2_s[:, :1], axis=0))
            e_cols = row_to_cols(ge[0:1, :], 4, "ecols")
            e_at = e_cols if wdt_at == F32 else cast_cols(e_cols, "ecat")
            hkc_at = hkc if wdt_at == F32 else cast_cols(hkc, "hkcat")
            e_cg = split_cols(e_cols, "ecg")
            hf_cg = split_cols(hfkc, "hfcg")
            hb_cg = split_cols(hbkc, "hbcg")

            # ------- GRU h-side partials (independent of x; run early) -------
            with tc.tile_pool(name="s0w", bufs=3) as s0w, \
                 tc.tile_pool(name="rows0", bufs=2) as rows0, \
                 tc.tile_pool(name="s0p", bufs=2, space="PSUM") as s0p:
                for di, (whrz_d, hcw) in enumerate(((whrzf_d, hf_cg), (whrzb_d, hb_cg))):
                    for c in range(8):
                        wh = s0w.tile([128, nv, 2, 512], wdt, name="wh", tag="wh")
                        nc.sync.dma_start(wh[:], whrz_d[:, c])
                        psH = s0p.tile([1, 512], F32, name="psH", tag="psH")
                        mm_group(psH[:], [(hcw, wh, 2)], True, True)
                        rb0 = rows0.tile([1, 512], F32, name="rb0", tag="rb0")
                        nc.scalar.copy(rb0[:], psH[:])
                        nc.gpsimd.dma_start(cc3_in[16 + 8 * di + c:17 + 8 * di + c, :], rb0[:])
                for di, (whn_d, hcw) in enumerate(((whnf_d, hf_cg), (whnb_d, hb_cg))):
                    for c in range(4):
                        wh2 = s0w.tile([128, nv, 2, 512], wdt, name="wh2", tag="wh")
                        nc.sync.dma_start(wh2[:], whn_d[:, c])
                        psH2 = s0p.tile([1, 512], F32, name="psH2", tag="psH")
                        mm_group(psH2[:], [(hcw, wh2, 2)], True, True)
                        rb0 = rows0.tile([1, 512], F32, name="rb0b", tag="rb0")
                        nc.scalar.copy(rb0[:], psH2[:])
                        nc.gpsimd.dma_start(cc3_in[40 + 4 * di + c:41 + 4 * di + c, :], rb0[:])

            # ---- prefetch first wout chunks into the idle barrier window ----
            wo_tiles = {}
            for c in range(PF):
                wo = s5w.tile([128, 32, 512], wdt, name="wo", tag="wo")
                nc.sync.dma_start(wo[:], wout_d[:, c])
                wo_tiles[c] = wo

            # ---------------- stage 1: attn logits partial ----------------
            with tc.tile_pool(name="s1p", bufs=1, space="PSUM") as s1p:
                psL = s1p.tile([1, 512], F32)
                for t in range(4):
                    _mm(nc, psL[:], e_at[:, t:t + 1], a1[:, t, :],
                        t == 0, False, dt_mode)
                for t in range(4):
                    _mm(nc, psL[:], hkc_at[:, t:t + 1], a2[:, t, :],
                        False, t == 3, dt_mode)
                sL = vp.tile([1, 512], F32)
                nc.scalar.copy(sL[:], psL[:])
                nc.gpsimd.dma_start(cc1_in[:], sL[:])

            nc.gpsimd.collective_compute("AllReduce", ALU.add, replica_groups=RG,
                                         ins=[cc1_in[:]], outs=[cc1_out[:]])

            # ---------------- softmax (replicated) ----------------
            lg = vp.tile([4, 128], F32)
            nc.gpsimd.dma_start(lg[:], cc1_out[:].rearrange("a (p t) -> (a p) t", t=128))
            nc.vector.tensor_tensor(lg[:], lg[:], abcm[:], op=ALU.add)
            exw = vp.tile([4, 128], F32)
            exs = vp.tile([4, 1], F32)
            nc.scalar.activation(exw[:], lg[:], AF.Exp, accum_out=exs[:])
            psZ = psv.tile([1, 1], F32, name="psZ", tag="psmall")
            nc.tensor.matmul(psZ[:], exs[:], ones4[:], start=True, stop=True)
            sZ = vp.tile([1, 1], F32)
            nc.scalar.copy(sZ[:], psZ[:])
            rZ = vp.tile([1, 1], F32)
            nc.vector.reciprocal(rZ[:], sZ[:])
            psR = psv.tile([4, 1], F32, name="psR", tag="psmall")
            nc.tensor.matmul(psR[:], ones14[:], rZ[:], start=True, stop=True)
            rZ4 = vp.tile([4, 1], F32)
            nc.scalar.copy(rZ4[:], psR[:])
            w_cm = vp.tile([4, 128], F32)
            nc.vector.tensor_scalar(w_cm[:], exw[:], rZ4[:, :1], None, ALU.mult)
            nc.gpsimd.dma_start(aw_d[:], w_cm[:])
            psWc = psv.tile([128, 4], F32, name="psWc", tag="ptrans")
            nc.tensor.matmul(psWc[:], w_cm[:], id32[0:4, 0:4], is_transpose=True,
                             start=True, stop=True)
            w_cols = vp.tile([128, 4], F32)
            nc.vector.tensor_copy(w_cols[:], psWc[:])
            w_at = w_cols if wdt_at == F32 else cast_cols(w_cols, "wat")

            # ---------------- stage 2: attn_applied slice ----------------
            with tc.tile_pool(name="s2p", bufs=1, space="PSUM") as s2p:
                psA = s2p.tile([1, 512], F32)
                for t in range(4):
                    _mm(nc, psA[:], w_at[:, t:t + 1], lat[:, t, :],
                        t == 0, t == 3, dt_mode)
                sA = vp.tile([1, 512], F32)
                nc.scalar.copy(sA[:], psA[:])
            a_cols = row_to_cols(sA, 4, "acols")
            a_cg = split_cols(a_cols, "acg")
            s1w_cm.__exit__(None, None, None)

            # ---------------- stage 3: comb partial ----------------
            with tc.tile_pool(name="s3w", bufs=(6 if wdt == BF16 and nv == 1 else 3)) as s3w, \
                 tc.tile_pool(name="rows3", bufs=2) as rows3, \
                 tc.tile_pool(name="s3p", bufs=2, space="PSUM") as s3p:
                for c in range(8):
                    c1t = s3w.tile([128, nv, 4, 512], wdt, name="c1t", tag="ct")
                    nc.sync.dma_start(c1t[:], c1_d[:, c])
                    c2t = s3w.tile([128, nv, 4, 512], wdt, name="c2t", tag="ct")
                    nc.sync.dma_start(c2t[:], c2_d[:, c])
                    psC = s3p.tile([1, 512], F32, name="psC", tag="psC")
                    mm_group(psC[:], [(e_cg, c1t, 4), (a_cg, c2t, 4)], True, True)
                    rb3 = rows3.tile([1, 512], F32, name="rb3", tag="rb3")
                    nc.scalar.copy(rb3[:], psC[:])
                    nc.gpsimd.dma_start(cc2_in[c:c + 1, :], rb3[:])

            nc.gpsimd.collective_compute("AllReduce", ALU.add, replica_groups=RG,
                                         ins=[cc2_in[:]], outs=[cc2_out[:]])

            # core-local x slice readback (rows 4k..4k+3 of [32,128] view)
            xr4 = vp.tile([4, 128], F32)
            nc.gpsimd.indirect_dma_start(
                out=xr4[:], out_offset=None,
                in_=cc2_out[:].rearrange("a (p t) -> (a p) t", t=128),
                in_offset=bass.IndirectOffsetOnAxis(ap=xrows_s[:, :1], axis=0))
            nc.vector.tensor_tensor(xr4[:], xr4[:], cbcm[:], op=ALU.add)
            xrelu = vp.tile([4, 128], F32)
            nc.scalar.activation(xrelu[:], xr4[:], AF.Relu)
            psXc = psv.tile([128, 4], F32, name="psXc", tag="ptrans")
            nc.tensor.matmul(psXc[:], xrelu[:], id32[0:4, 0:4], is_transpose=True,
                             start=True, stop=True)
            xk_cols = vp.tile([128, 4], F32)
            nc.vector.tensor_copy(xk_cols[:], psXc[:])
            xk_cg = split_cols(xk_cols, "xkcg")

            # ---------------- stage 4: GRU gate partials ----------------
            # gates_sb rows (512 each): 0:8 rz_f | 8:16 rz_b | 16:20 in_f |
            # 20:24 in_b | 24:28 hn_f | 28:32 hn_b
            with tc.tile_pool(name="s4w", bufs=(6 if wdt == BF16 and nv == 1 else 3)) as s4w, \
                 tc.tile_pool(name="rows4", bufs=2) as rows4, \
                 tc.tile_pool(name="s4p", bufs=2, space="PSUM") as s4p:
                for di, wrz_d in enumerate((wrzf_d, wrzb_d)):
                    for c in range(8):
                        wt = s4w.tile([128, nv, 4, 512], wdt, name="wrzt", tag="gt")
                        nc.sync.dma_start(wt[:], wrz_d[:, c])
                        psG = s4p.tile([1, 512], F32, name="psG", tag="psG")
                        mm_group(psG[:], [(xk_cg, wt, 4)], True, True)
                        r = 8 * di + c
                        rb4 = rows4.tile([1, 512], F32, name="rb4", tag="rb4")
                        nc.scalar.copy(rb4[:], psG[:])
                        nc.gpsimd.dma_start(cc3_in[r:r + 1, :], rb4[:])
                for di, win_d in enumerate((winf_d, winb_d)):
                    for c in range(4):
                        wt2 = s4w.tile([128, nv, 4, 512], wdt, name="wint", tag="gt")
                        nc.sync.dma_start(wt2[:], win_d[:, c])
                        psG2 = s4p.tile([1, 512], F32, name="psG2", tag="psG")
                        mm_group(psG2[:], [(xk_cg, wt2, 4)], True, True)
                        r = 32 + 4 * di + c
                        rb4 = rows4.tile([1, 512], F32, name="rb4b", tag="rb4")
                        nc.scalar.copy(rb4[:], psG2[:])
                        nc.gpsimd.dma_start(cc3_in[r:r + 1, :], rb4[:])
            nc.gpsimd.collective_compute("AllReduce", ALU.add, replica_groups=RG,
                                         ins=[cc3_in[:]], outs=[cc3_out[:]])

            # ---------------- gate math (replicated) ----------------
            # cc3 buf offsets: r_f 0:2048 | z_f 2048:4096 | r_b 4096:6144 |
            # z_b 6144:8192 | in_f 8192:10240 | in_b 10240:12288 |
            # hn_f 12288:14336 | hn_b 14336:16384
            # read back as [32,128] tiles, rows 0:16 = f, 16:32 = b
            # buf rows(512): 0:8 rzx_f | 8:16 rzx_b | 16:24 rzh_f | 24:32 rzh_b
            #                | 32:36 in_f | 36:40 in_b | 40:44 hn_f | 44:48 hn_b
            cc3v = cc3_out[:].rearrange("a b -> (a b)").rearrange("(p t) -> p t", t=128)
            rt = vp.tile([32, 128], F32)
            nc.gpsimd.dma_start(rt[0:16, :], cc3v[0:16, :])
            nc.gpsimd.dma_start(rt[16:32, :], cc3v[32:48, :])
            zt_ = vp.tile([32, 128], F32)
            nc.gpsimd.dma_start(zt_[0:16, :], cc3v[16:32, :])
            nc.gpsimd.dma_start(zt_[16:32, :], cc3v[48:64, :])
            rh = vp.tile([32, 128], F32)
            nc.gpsimd.dma_start(rh[0:16, :], cc3v[64:80, :])
            nc.gpsimd.dma_start(rh[16:32, :], cc3v[96:112, :])
            zh = vp.tile([32, 128], F32)
            nc.gpsimd.dma_start(zh[0:16, :], cc3v[80:96, :])
            nc.gpsimd.dma_start(zh[16:32, :], cc3v[112:128, :])
            it_ = vp.tile([32, 128], F32)
            nc.gpsimd.dma_start(it_[:], cc3v[128:160, :])
            ht_ = vp.tile([32, 128], F32)
            nc.gpsimd.dma_start(ht_[:], cc3v[160:192, :])
            nc.vector.tensor_tensor(rt[:], rt[:], rh[:], op=ALU.add)
            nc.vector.tensor_tensor(zt_[:], zt_[:], zh[:], op=ALU.add)
            # biases, same split (host supplies matching layouts)
            nc.vector.tensor_tensor(rt[:], rt[:], gbr[:], op=ALU.add)
            nc.vector.tensor_tensor(zt_[:], zt_[:], gbz[:], op=ALU.add)
            nc.vector.tensor_tensor(it_[:], it_[:], gbi[:], op=ALU.add)
            nc.vector.tensor_tensor(ht_[:], ht_[:], gbh[:], op=ALU.add)
            sigr = vp.tile([32, 128], F32)
            nc.scalar.activation(sigr[:], rt[:], AF.Sigmoid)
            sigz = vp.tile([32, 128], F32)
            nc.scalar.activation(sigz[:], zt_[:], AF.Sigmoid)
            narg = vp.tile([32, 128], F32)
            nc.vector.tensor_tensor(narg[:], sigr[:], ht_[:], op=ALU.mult)
            nc.vector.tensor_tensor(narg[:], narg[:], it_[:], op=ALU.add)
            ngate = vp.tile([32, 128], F32)
            nc.scalar.activation(ngate[:], narg[:], AF.Tanh)
            dlt = vp.tile([32, 128], F32)
            nc.vector.tensor_tensor(dlt[:], hprev[:], ngate[:], op=ALU.subtract)
            zd = vp.tile([32, 128], F32)
            nc.vector.tensor_tensor(zd[:], sigz[:], dlt[:], op=ALU.mult)
            h_new = vp.tile([32, 128], F32)
            nc.vector.tensor_tensor(h_new[:], ngate[:], zd[:], op=ALU.add)
            nc.gpsimd.dma_start(nh_d[:], h_new[:])
            psGc = psv.tile([128, 32], F32, name="psGc", tag="ptrans")
            nc.tensor.matmul(psGc[:], h_new[:], id32[:, :], is_transpose=True,
                             start=True, stop=True)
            gru_cols = vp.tile([128, 32], F32)
            nc.vector.tensor_copy(gru_cols[:], psGc[:])
            gru_colsw = gru_cols if wdt == F32 else cast_cols(gru_cols, "grucols")

            # ---------------- stage 5: vocab projection ----------------
            ldram = dr.tile([13, 512], F32)
            with tc.tile_pool(name="rows5", bufs=2) as rows5, \
                 tc.tile_pool(name="s5p", bufs=2, space="PSUM") as s5p:
                for c in range(13):
                    if c in wo_tiles:
                        wo = wo_tiles[c]
                    else:
                        wo = s5w.tile([128, 32, 512], wdt, name="wo", tag="wo")
                        nc.sync.dma_start(wo[:], wout_d[:, c])
                    psO = s5p.tile([1, 512], F32, name="psO", tag="psO")
                    for t in range(32):
                        _mm(nc, psO[:], gru_colsw[:, t:t + 1], wo[:, t, :],
                            t == 0, t == 31, dt_mode)
                    rb5 = rows5.tile([1, 512], F32, name="rb5", tag="rb5")
                    nc.scalar.copy(rb5[:], psO[:])
                    nc.gpsimd.dma_start(ldram[c:c + 1, :], rb5[:])
            logits_sb = vp.tile([13, 512], F32)
            nc.gpsimd.dma_start(logits_sb[:], ldram[:])

            nc.vector.tensor_tensor(logits_sb[:], logits_sb[:], obcm[:], op=ALU.add)
            exo = vp.tile([13, 512], F32)
            se13 = vp.tile([13, 1], F32)
            nc.scalar.activation(exo[:], logits_sb[:], AF.Exp, accum_out=se13[:])
            psZ2 = psv.tile([1, 1], F32, name="psZ2", tag="psmall")
            nc.tensor.matmul(psZ2[:], se13[:], ones13[:], start=True, stop=True)
            seZ = vp.tile([1, 1], F32)
            nc.scalar.copy(seZ[:], psZ2[:])
            nc.gpsimd.dma_start(cc4_in[:], seZ[:])
            nc.gpsimd.collective_compute("AllReduce", ALU.add, replica_groups=RG,
                                         ins=[cc4_in[:]], outs=[cc4_out[:]])
            zt = vp.tile([1, 1], F32)
            nc.gpsimd.dma_start(zt[:], cc4_out[:])
            lnz = vp.tile([1, 1], F32)
            nc.scalar.activation(lnz[:], zt[:], AF.Ln)
            nlz = vp.tile([1, 1], F32)
            nc.vector.tensor_scalar(nlz[:], lnz[:], -1.0, None, ALU.mult)
            psB = psv.tile([13, 1], F32, name="psB", tag="psmall")
            nc.tensor.matmul(psB[:], ones113[:], nlz[:], start=True, stop=True)
            nlz13 = vp.tile([13, 1], F32)
            nc.scalar.copy(nlz13[:], psB[:])
            logp_sb = vp.tile([13, 512], F32)
            nc.scalar.activation(logp_sb[:], logits_sb[:], AF.Identity,
                                 bias=nlz13[:, :1])
            nc.gpsimd.dma_start(lp_d[:], logp_sb[:])

    nc.compile()
    return nc


def _tiles(mat, nt, nch):
    """[nt*128, nch*512] -> [128, nch, nt, 512] host blocking."""
    r, c = mat.shape
    assert r == nt * 128 and c == nch * 512, (mat.shape, nt, nch)
    return np.ascontiguousarray(
        mat.reshape(nt, 128, nch, 512).transpose(1, 2, 0, 3))


def _prep_inputs(inputs):
    f32 = np.float32
    input_ids = np.asarray(inputs["input_ids"])
    hidden = np.asarray(inputs["hidden"], f32)
    latent_out = np.asarray(inputs["latent_out"], f32)
    emb = np.asarray(inputs["emb"], f32)
    attn_W = np.asarray(inputs["attn_W"], f32)
    attn_b = np.asarray(inputs["attn_b"], f32)
    comb_W = np.asarray(inputs["comb_W"], f32)
    comb_b = np.asarray(inputs["comb_b"], f32)
    out_W = np.asarray(inputs["out_W"], f32)
    out_b = np.asarray(inputs["out_b"], f32)
    Wih = {"f": np.asarray(inputs["Wih_f"], f32), "b": np.asarray(inputs["Wih_b"], f32)}
    Whh = {"f": np.asarray(inputs["Whh_f"], f32), "b": np.asarray(inputs["Whh_b"], f32)}
    bih = {"f": np.asarray(inputs["bih_f"], f32), "b": np.asarray(inputs["bih_b"], f32)}
    bhh = {"f": np.asarray(inputs["bhh_f"], f32), "b": np.asarray(inputs["bhh_b"], f32)}

    idx = int(np.asarray(input_ids).reshape(-1)[0])
    hflat = hidden.reshape(-1)                      # [4096] = [h_f; h_b]
    aWT = np.ascontiguousarray(attn_W.T)            # [8192, 512]
    cWT = np.ascontiguousarray(comb_W.T)            # [8192, 4096]
    WihT = {d: np.ascontiguousarray(Wih[d].T) for d in "fb"}   # [4096, 6144]
    WhhT = {d: np.ascontiguousarray(Whh[d].T) for d in "fb"}   # [2048, 6144]
    oWTp = np.zeros((H, VP), f32)                   # [4096, 53248]
    oWTp[:, :V] = out_W.T
    obp = np.full((VP,), NEG, f32)
    obp[:V] = out_b
    # gate biases as [32,128] (rows 0:16 fwd, 16:32 bwd)
    bsum = {d: bih[d] + bhh[d] for d in "fb"}
    gbr = np.concatenate([bsum["f"][0:2048], bsum["b"][0:2048]]).reshape(32, 128)
    gbz = np.concatenate([bsum["f"][2048:4096], bsum["b"][2048:4096]]).reshape(32, 128)
    gbi = np.concatenate([bih["f"][4096:6144], bih["b"][4096:6144]]).reshape(32, 128)
    gbh = np.concatenate([bhh["f"][4096:6144], bhh["b"][4096:6144]]).reshape(32, 128)

    if DT_MODE in ("bf16", "mixed"):
        import ml_dtypes
        bfc = lambda a: np.ascontiguousarray(a).astype(ml_dtypes.bfloat16)
    if DT_MODE == "bf16":
        wcast = bfc                     # attn group too
        acast = bfc
        def gcast(a):                   # comb/gru: add nv=1 axis
            return bfc(a)[:, :, None]
    elif DT_MODE == "mixed":
        acast = lambda a: a             # attn stays f32
        wcast = bfc                     # out projection bf16
        def gcast(a):                   # comb/gru: (hi, lo) pair axis
            hi = bfc(a)
            lo = bfc(a - hi.astype(np.float32))
            return np.ascontiguousarray(np.stack([hi, lo], axis=2))
    else:
        wcast = lambda a: a
        acast = lambda a: a
        gcast = lambda a: np.ascontiguousarray(a)[:, :, None]

    eye32 = np.eye(32, dtype=f32)
    in_maps = []
    for k in range(M):
        s = 512 * k
        hs = 256 * k
        im = {
            "embp": np.ascontiguousarray(emb[:, s:s + 512]),
            "idx2": np.array([[idx], [idx]], np.int32),
            "xrows": (4 * k + np.arange(4, dtype=np.int32)).reshape(4, 1),
            "id32": eye32,
            "hkc": np.ascontiguousarray(hflat[s:s + 512].reshape(4, 128).T),
            "hfkc": np.ascontiguousarray(hidden[0, 0, hs:hs + 256].reshape(2, 128).T),
            "hbkc": np.ascontiguousarray(hidden[1, 0, hs:hs + 256].reshape(2, 128).T),
            "hprev": np.ascontiguousarray(hidden.reshape(32, 128)),
            "abcm": attn_b.reshape(4, 128),
            "cbcm": np.ascontiguousarray(comb_b[s:s + 512].reshape(4, 128)),
            "gbr": gbr, "gbz": gbz, "gbi": gbi, "gbh": gbh,
            "obcm": np.ascontiguousarray(obp[VC * k:VC * (k + 1)].reshape(13, 512)),
            "a1": acast(_tiles(aWT[s:s + 512], 4, 1)),
            "a2": acast(_tiles(aWT[4096 + s:4096 + s + 512], 4, 1)),
            "lat": acast(_tiles(np.ascontiguousarray(latent_out[:, s:s + 512]), 4, 1)),
            "c1": gcast(_tiles(cWT[s:s + 512], 4, 8)),
            "c2": gcast(_tiles(cWT[4096 + s:4096 + s + 512], 4, 8)),
            "wout": wcast(_tiles(np.ascontiguousarray(oWTp[:, VC * k:VC * (k + 1)]),
                                 32, 13)),
        }
        for d in "fb":
            im[f"wrz{d}"] = gcast(_tiles(WihT[d][s:s + 512, 0:4096], 4, 8))
            im[f"whrz{d}"] = gcast(_tiles(WhhT[d][hs:hs + 256, 0:4096], 2, 8))
            im[f"win{d}"] = gcast(_tiles(WihT[d][s:s + 512, 4096:6144], 4, 4))
            im[f"whn{d}"] = gcast(_tiles(WhhT[d][hs:hs + 256, 4096:6144], 2, 4))
        in_maps.append(im)
    return in_maps


def run(inputs, trace=False):
    if DT_MODE not in _CACHE:
        _CACHE[DT_MODE] = _build(DT_MODE)
    nc = _CACHE[DT_MODE]
    in_maps = _prep_inputs(inputs)
    res = run_bass_kernel_spmd(nc, in_maps, core_ids=list(range(M)), trace=trace)
    logp = np.concatenate([res.results[c]["lp"].reshape(-1) for c in range(M)])
    logp = logp[:V].reshape(1, V)
    new_hidden = res.results[0]["nh"].reshape(2, 1, Hh)
    attn_weights = res.results[0]["aw"].reshape(1, L)
    return (logp, new_hidden, attn_weights), res


def kernel(**inputs):
    out, _ = run(inputs, trace=False)
    return out


# revision 23
# speedup vs baseline: 1.0393x; 1.0393x over previous
"""AttnDecoderRNN step on 8 TRN2 NeuronCores, tensor-parallel.

Strategy (contraction sharding + AllReduce between stages):
  stage1 attn logits : attn_W.T row-sharded (512 emb rows + 512 hid rows
                       per core), partial [512] -> AR -> +bias -> softmax
  stage2 attn_applied: latent_out column-sharded; each core computes its
                       own 512-wide slice (no collective)
  stage3 comb        : comb_W.T row-sharded (512 emb + 512 attn rows per
                       core), partial [4096] -> AR -> slice readback ->
                       +bias -> relu -> x slice
  stage4 GRU gates   : Wih/Whh contraction-sharded (512 x-rows, 256 h-rows
                       per core); r/z gates pre-summed (gi+gh) in PSUM, the
                       n gate halves kept split; partial [16384] -> AR ->
                       +bias -> gate math -> h_new
  stage5 vocab proj  : out_W output-sharded (6656 rows per core, padded to
                       53248); +bias, local sumexp -> AR(scalar) -> logp
Embedding lookup: emb column-sharded; one indirect-DMA row gather per core.
All weights are pre-transposed/tiled on the host so every DMA is contiguous
and every matmul operand sits at partition base 0.  Matvec form:
psum[1,N] += lhsT(x column tile [128,1]).T @ rhs(W.T tile [128,N]).
Biases are applied once, post-AllReduce, as elementwise adds.
"""
import os
import sys

sys.path.insert(0, "/opt/trn_rl_repo")

import numpy as np

from concourse import bass, bacc, tile, mybir
from concourse.bass_utils import run_bass_kernel_spmd

V, H, L, Hh = 50257, 4096, 512, 2048
M = 8               # cores
VC = 6656           # per-core padded vocab rows (13 * 512)
VP = VC * M         # 53248
NEG = -1.0e30

F32 = mybir.dt.float32
F32R = mybir.dt.float32r
BF16 = mybir.dt.bfloat16
I32 = mybir.dt.int32

# 'f32' (exact, PE 4cyc/row) or 'bf16' (half DMA bytes, PE 1cyc/row)
DT_MODE = os.environ.get("BASS_KERNEL_DT", "f32")

AF = mybir.ActivationFunctionType
ALU = mybir.AluOpType
RG = [list(range(M))]

_CACHE = {}


def _mm(nc, out, lhsT, rhs, start, stop, dt_mode):
    if dt_mode == "f32r":
        lhsT = lhsT.bitcast(F32R)
        rhs = rhs.bitcast(F32R)
    nc.tensor.matmul(out, lhsT, rhs, start=start, stop=stop)


def _build(dt_mode):
    wdt = {"f32": F32, "f32r": F32, "bf16": BF16, "mixed": BF16}[dt_mode]
    wdt_at = F32 if dt_mode in ("f32", "f32r", "mixed") else BF16
    pair = dt_mode == "mixed"
    nv = 2 if pair else 1
    nc = bacc.Bacc("TRN2", target_bir_lowering=False, debug=False,
                   enable_asserts=True, num_devices=M)

    def din(name, shape, dt=F32):
        return nc.dram_tensor(name, shape, dt, kind="ExternalInput").ap()

    def dout(name, shape, dt=F32):
        return nc.dram_tensor(name, shape, dt, kind="ExternalOutput").ap()

    embp = din("embp", [V, 512])
    idx2 = din("idx2", [2, 1], I32)
    xrows = din("xrows", [4, 1], I32)
    id32_d = din("id32", [32, 32])
    hkc_d = din("hkc", [128, 4])
    hfkc_d = din("hfkc", [128, 2])
    hbkc_d = din("hbkc", [128, 2])
    hprev_d = din("hprev", [32, 128])
    abcm_d = din("abcm", [4, 128])
    cbcm_d = din("cbcm", [4, 128])
    gbr_d = din("gbr", [32, 128])
    gbz_d = din("gbz", [32, 128])
    gbi_d = din("gbi", [32, 128])
    gbh_d = din("gbh", [32, 128])
    obf_d = din("obf", [1, VC])
    a1_d = din("a1", [128, 4, 512], wdt_at)
    a2_d = din("a2", [128, 4, 512], wdt_at)
    lat_d = din("lat", [128, 4, 512], wdt_at)
    c1_d = din("c1", [128, 8, nv, 4, 512], wdt)
    c2_d = din("c2", [128, 8, nv, 4, 512], wdt)
    wrzf_d = din("wrzf", [128, 8, nv, 4, 512], wdt)
    wrzb_d = din("wrzb", [128, 8, nv, 4, 512], wdt)
    whrzf_d = din("whrzf", [128, 8, nv, 2, 512], wdt)
    whrzb_d = din("whrzb", [128, 8, nv, 2, 512], wdt)
    winf_d = din("winf", [128, 4, nv, 4, 512], wdt)
    winb_d = din("winb", [128, 4, nv, 4, 512], wdt)
    whnf_d = din("whnf", [128, 4, nv, 2, 512], wdt)
    whnb_d = din("whnb", [128, 4, nv, 2, 512], wdt)
    wout_d = din("wout", [128, 13, 32, 512], wdt)

    lp_d = dout("lp", [13, 512])
    nh_d = dout("nh", [32, 128])
    aw_d = dout("aw", [4, 128])

    with tile.TileContext(nc) as tc:
        PF = (3 if pair else 4) if wdt == BF16 else 1
        with tc.tile_pool(name="const", bufs=1) as cp, \
             tc.tile_pool(name="vec", bufs=1) as vp, \
             tc.tile_pool(name="psv", bufs=2, space="PSUM") as psv, \
             tc.tile_pool(name="s5w", bufs=max(PF, 2)) as s5w, \
             tc.tile_pool(name="dram", bufs=1, space="DRAM") as dr:

            id32 = cp.tile([32, 32], F32)
            nc.sync.dma_start(id32[:], id32_d[:])
            ones4 = cp.tile([4, 1], F32)
            nc.vector.memset(ones4[:], 1.0)
            ones13 = cp.tile([13, 1], F32)
            nc.vector.memset(ones13[:], 1.0)
            ones113 = cp.tile([1, 13], F32)
            nc.vector.memset(ones113[:], 1.0)
            ones14 = cp.tile([1, 4], F32)
            nc.vector.memset(ones14[:], 1.0)

            idx2_s = cp.tile([2, 1], I32)
            nc.sync.dma_start(idx2_s[:], idx2[:])
            xrows_s = cp.tile([4, 1], I32)
            nc.sync.dma_start(xrows_s[:], xrows[:])
            hkc = cp.tile([128, 4], F32)
            nc.sync.dma_start(hkc[:], hkc_d[:])
            hfkc = cp.tile([128, 2], F32)
            nc.sync.dma_start(hfkc[:], hfkc_d[:])
            hbkc = cp.tile([128, 2], F32)
            nc.sync.dma_start(hbkc[:], hbkc_d[:])
            hprev = cp.tile([32, 128], F32)
            nc.sync.dma_start(hprev[:], hprev_d[:])
            abcm = cp.tile([4, 128], F32)
            nc.sync.dma_start(abcm[:], abcm_d[:])
            cbcm = cp.tile([4, 128], F32)
            nc.sync.dma_start(cbcm[:], cbcm_d[:])
            gbr = cp.tile([32, 128], F32)
            nc.sync.dma_start(gbr[:], gbr_d[:])
            gbz = cp.tile([32, 128], F32)
            nc.sync.dma_start(gbz[:], gbz_d[:])
            gbi = cp.tile([32, 128], F32)
            nc.sync.dma_start(gbi[:], gbi_d[:])
            gbh = cp.tile([32, 128], F32)
            nc.sync.dma_start(gbh[:], gbh_d[:])

            s1w_cm = tc.tile_pool(name="s1w", bufs=1)
            s1w = s1w_cm.__enter__()
            a1 = s1w.tile([128, 4, 512], wdt_at)
            nc.sync.dma_start(a1[:], a1_d[:])
            a2 = s1w.tile([128, 4, 512], wdt_at)
            nc.sync.dma_start(a2[:], a2_d[:])
            lat = s1w.tile([128, 4, 512], wdt_at)
            nc.sync.dma_start(lat[:], lat_d[:])

            # collective bounce buffers
            cc1_in = dr.tile([1, 512], F32)
            cc1_out = dr.tile([1, 512], F32, addr_space="Shared")
            cc2_in = dr.tile([8, 512], F32)
            cc2_out = dr.tile([8, 512], F32, addr_space="Shared")
            cc3_in = dr.tile([48, 512], F32)
            cc3_out = dr.tile([48, 512], F32, addr_space="Shared")
            cc4_in = dr.tile([1, 1], F32)
            cc4_out = dr.tile([1, 1], F32, addr_space="Shared")

            def row_to_cols(row, n, name):
                """[1, 128*n] SBUF row -> [128, n] col tiles via PE transposes."""
                pc = psv.tile([128, n], F32, name=f"{name}_ps", tag="ptrans")
                for t in range(n):
                    nc.tensor.matmul(pc[:, t:t + 1], row[0:1, bass.ts(t, 128)],
                                     id32[0:1, 0:1], is_transpose=True,
                                     start=(t == 0), stop=(t == n - 1))
                out = vp.tile([128, n], F32, name=f"{name}_sb", tag=f"{name}_sb")
                nc.vector.tensor_copy(out[:], pc[:])
                return out

            def cast_cols(cols, name):
                if wdt != BF16:
                    return cols
                cb = vp.tile(list(cols.shape), BF16, name=f"{name}_bf",
                             tag=f"{name}_bf")
                nc.vector.tensor_copy(cb[:], cols[:])
                return cb

            def split_cols(cols, name):
                """f32 cols -> (hi, lo) bf16 pair; or single bf16/f32 view."""
                if not pair:
                    if wdt == BF16:
                        cb = vp.tile(list(cols.shape), BF16, name=f"{name}_h",
                                     tag=f"{name}_h")
                        nc.vector.tensor_copy(cb[:], cols[:])
                        return (cb,)
                    return (cols,)
                hi = vp.tile(list(cols.shape), BF16, name=f"{name}_h",
                             tag=f"{name}_h")
                nc.vector.tensor_copy(hi[:], cols[:])
                hif = vp.tile(list(cols.shape), F32, name=f"{name}_hf",
                              tag=f"{name}_hf")
                nc.vector.tensor_copy(hif[:], hi[:])
                dif = vp.tile(list(cols.shape), F32, name=f"{name}_d",
                              tag=f"{name}_d")
                nc.vector.tensor_tensor(dif[:], cols[:], hif[:], op=ALU.subtract)
                lo = vp.tile(list(cols.shape), BF16, name=f"{name}_l",
                             tag=f"{name}_l")
                nc.vector.tensor_copy(lo[:], dif[:])
                return (hi, lo)

            def mm_group(ps, lhs_sets, first, last):
                """Accumulate sum_j lhs_sets[j] @ tiles_j into ps.
                lhs_sets: list of (cols_tuple, wtile_ap, nt); wtile_ap indexed
                [v, t] in pair mode else [t]."""
                seq = []
                for cols, wtile, nt in lhs_sets:
                    for t in range(nt):
                        seq.append((cols[0][:, t:t + 1], wtile[:, 0, t, :]))
                        if pair:
                            seq.append((cols[1][:, t:t + 1], wtile[:, 0, t, :]))
                            seq.append((cols[0][:, t:t + 1], wtile[:, 1, t, :]))
                for i, (l, r) in enumerate(seq):
                    nc.tensor.matmul(ps, l, r,
                                     start=(first and i == 0),
                                     stop=(last and i == len(seq) - 1))

            # ---------------- embedding gather ----------------
            ge = vp.tile([2, 512], F32)
            nc.gpsimd.indirect_dma_start(
                out=ge[:], out_offset=None, in_=embp[:],
                in_offset=bass.IndirectOffsetOnAxis(ap=idx2_s[:, :1], axis=0))
            e_cols = row_to_cols(ge[0:1, :], 4, "ecols")
            e_at = e_cols if wdt_at == F32 else cast_cols(e_cols, "ecat")
            hkc_at = hkc if wdt_at == F32 else cast_cols(hkc, "hkcat")
            e_cg = split_cols(e_cols, "ecg")
            hf_cg = split_cols(hfkc, "hfcg")
            hb_cg = split_cols(hbkc, "hbcg")

            # ------- GRU h-side partials (independent of x; run early) -------
            with tc.tile_pool(name="s0w", bufs=3) as s0w, \
                 tc.tile_pool(name="rows0", bufs=2) as rows0, \
                 tc.tile_pool(name="s0p", bufs=2, space="PSUM") as s0p:
                for di, (whrz_d, hcw) in enumerate(((whrzf_d, hf_cg), (whrzb_d, hb_cg))):
                    for c in range(8):
                        wh = s0w.tile([128, nv, 2, 512], wdt, name="wh", tag="wh")
                        nc.sync.dma_start(wh[:], whrz_d[:, c])
                        psH = s0p.tile([1, 512], F32, name="psH", tag="psH")
                        mm_group(psH[:], [(hcw, wh, 2)], True, True)
                        rb0 = rows0.tile([1, 512], F32, name="rb0", tag="rb0")
                        nc.scalar.copy(rb0[:], psH[:])
                        nc.scalar.dma_start(cc3_in[16 + 8 * di + c:17 + 8 * di + c, :], rb0[:])
                for di, (whn_d, hcw) in enumerate(((whnf_d, hf_cg), (whnb_d, hb_cg))):
                    for c in range(4):
                        wh2 = s0w.tile([128, nv, 2, 512], wdt, name="wh2", tag="wh")
                        nc.sync.dma_start(wh2[:], whn_d[:, c])
                        psH2 = s0p.tile([1, 512], F32, name="psH2", tag="psH")
                        mm_group(psH2[:], [(hcw, wh2, 2)], True, True)
                        rb0 = rows0.tile([1, 512], F32, name="rb0b", tag="rb0")
                        nc.scalar.copy(rb0[:], psH2[:])
                        nc.scalar.dma_start(cc3_in[40 + 4 * di + c:41 + 4 * di + c, :], rb0[:])

            # ---------------- stage 1: attn logits partial ----------------
            with tc.tile_pool(name="s1p", bufs=1, space="PSUM") as s1p:
                psL = s1p.tile([1, 512], F32)
                for t in range(4):
                    _mm(nc, psL[:], e_at[:, t:t + 1], a1[:, t, :],
                        t == 0, False, dt_mode)
                for t in range(4):
                    _mm(nc, psL[:], hkc_at[:, t:t + 1], a2[:, t, :],
                        False, t == 3, dt_mode)
                sL = vp.tile([1, 512], F32)
                nc.scalar.copy(sL[:], psL[:])
                nc.scalar.dma_start(cc1_in[:], sL[:])

            nc.gpsimd.collective_compute("AllReduce", ALU.add, replica_groups=RG,
                                         ins=[cc1_in[:]], outs=[cc1_out[:]])

            # ---------------- softmax (replicated) ----------------
            lg = vp.tile([4, 128], F32)
            nc.gpsimd.dma_start(lg[:], cc1_out[:].rearrange("a (p t) -> (a p) t", t=128))
            nc.vector.tensor_tensor(lg[:], lg[:], abcm[:], op=ALU.add)
            exw = vp.tile([4, 128], F32)
            exs = vp.tile([4, 1], F32)
            nc.scalar.activation(exw[:], lg[:], AF.Exp, accum_out=exs[:])
            psZ = psv.tile([1, 1], F32, name="psZ", tag="psmall")
            nc.tensor.matmul(psZ[:], exs[:], ones4[:], start=True, stop=True)
            sZ = vp.tile([1, 1], F32)
            nc.scalar.copy(sZ[:], psZ[:])
            rZ = vp.tile([1, 1], F32)
            nc.vector.reciprocal(rZ[:], sZ[:])
            psR = psv.tile([4, 1], F32, name="psR", tag="psmall")
            nc.tensor.matmul(psR[:], ones14[:], rZ[:], start=True, stop=True)
            rZ4 = vp.tile([4, 1], F32)
            nc.scalar.copy(rZ4[:], psR[:])
            w_cm = vp.tile([4, 128], F32)
            nc.vector.tensor_scalar(w_cm[:], exw[:], rZ4[:, :1], None, ALU.mult)
            nc.gpsimd.dma_start(aw_d[:], w_cm[:])
            psWc = psv.tile([128, 4], F32, name="psWc", tag="ptrans")
            nc.tensor.matmul(psWc[:], w_cm[:], id32[0:4, 0:4], is_transpose=True,
                             start=True, stop=True)
            w_cols = vp.tile([128, 4], F32)
            nc.vector.tensor_copy(w_cols[:], psWc[:])
            w_at = w_cols if wdt_at == F32 else cast_cols(w_cols, "wat")

            # ---------------- stage 2: attn_applied slice ----------------
            with tc.tile_pool(name="s2p", bufs=1, space="PSUM") as s2p:
                psA = s2p.tile([1, 512], F32)
                for t in range(4):
                    _mm(nc, psA[:], w_at[:, t:t + 1], lat[:, t, :],
                        t == 0, t == 3, dt_mode)
                sA = vp.tile([1, 512], F32)
                nc.scalar.copy(sA[:], psA[:])
            a_cols = row_to_cols(sA, 4, "acols")
            a_cg = split_cols(a_cols, "acg")
            s1w_cm.__exit__(None, None, None)

            # ---------------- stage 3: comb partial ----------------
            with tc.tile_pool(name="s3w", bufs=(6 if wdt == BF16 and nv == 1 else 3)) as s3w, \
                 tc.tile_pool(name="rows3", bufs=2) as rows3, \
                 tc.tile_pool(name="s3p", bufs=2, space="PSUM") as s3p:
                for c in range(8):
                    c1t = s3w.tile([128, nv, 4, 512], wdt, name="c1t", tag="ct")
                    nc.sync.dma_start(c1t[:], c1_d[:, c])
                    c2t = s3w.tile([128, nv, 4, 512], wdt, name="c2t", tag="ct")
                    nc.sync.dma_start(c2t[:], c2_d[:, c])
                    psC = s3p.tile([1, 512], F32, name="psC", tag="psC")
                    mm_group(psC[:], [(e_cg, c1t, 4), (a_cg, c2t, 4)], True, True)
                    rb3 = rows3.tile([1, 512], F32, name="rb3", tag="rb3")
                    nc.scalar.copy(rb3[:], psC[:])
                    nc.scalar.dma_start(cc2_in[c:c + 1, :], rb3[:])

            # prefetch first wout chunks while the attn/comb ARs run
            wo_tiles = {}
            for c in range(PF):
                wo = s5w.tile([128, 32, 512], wdt, name="wo", tag="wo")
                nc.sync.dma_start(wo[:], wout_d[:, c])
                wo_tiles[c] = wo

            nc.gpsimd.collective_compute("AllReduce", ALU.add, replica_groups=RG,
                                         ins=[cc2_in[:]], outs=[cc2_out[:]])

            # core-local x slice readback (rows 4k..4k+3 of [32,128] view)
            xr4 = vp.tile([4, 128], F32)
            nc.gpsimd.indirect_dma_start(
                out=xr4[:], out_offset=None,
                in_=cc2_out[:].rearrange("a (p t) -> (a p) t", t=128),
                in_offset=bass.IndirectOffsetOnAxis(ap=xrows_s[:, :1], axis=0))
            nc.vector.tensor_tensor(xr4[:], xr4[:], cbcm[:], op=ALU.add)
            xrelu = vp.tile([4, 128], F32)
            nc.scalar.activation(xrelu[:], xr4[:], AF.Relu)
            psXc = psv.tile([128, 4], F32, name="psXc", tag="ptrans")
            nc.tensor.matmul(psXc[:], xrelu[:], id32[0:4, 0:4], is_transpose=True,
                             start=True, stop=True)
            xk_cols = vp.tile([128, 4], F32)
            nc.vector.tensor_copy(xk_cols[:], psXc[:])
            xk_cg = split_cols(xk_cols, "xkcg")

            # ---------------- stage 4: GRU gate partials ----------------
            # gates_sb rows (512 each): 0:8 rz_f | 8:16 rz_b | 16:20 in_f |
            # 20:24 in_b | 24:28 hn_f | 28:32 hn_b
            with tc.tile_pool(name="s4w", bufs=(6 if wdt == BF16 and nv == 1 else 3)) as s4w, \
                 tc.tile_pool(name="rows4", bufs=2) as rows4, \
                 tc.tile_pool(name="s4p", bufs=2, space="PSUM") as s4p:
                for di, wrz_d in enumerate((wrzf_d, wrzb_d)):
                    for c in range(8):
                        wt = s4w.tile([128, nv, 4, 512], wdt, name="wrzt", tag="gt")
                        nc.sync.dma_start(wt[:], wrz_d[:, c])
                        psG = s4p.tile([1, 512], F32, name="psG", tag="psG")
                        mm_group(psG[:], [(xk_cg, wt, 4)], True, True)
                        r = 8 * di + c
                        rb4 = rows4.tile([1, 512], F32, name="rb4", tag="rb4")
                        nc.scalar.copy(rb4[:], psG[:])
                        nc.scalar.dma_start(cc3_in[r:r + 1, :], rb4[:])
                for di, win_d in enumerate((winf_d, winb_d)):
                    for c in range(4):
                        wt2 = s4w.tile([128, nv, 4, 512], wdt, name="wint", tag="gt")
                        nc.sync.dma_start(wt2[:], win_d[:, c])
                        psG2 = s4p.tile([1, 512], F32, name="psG2", tag="psG")
                        mm_group(psG2[:], [(xk_cg, wt2, 4)], True, True)
                        r = 32 + 4 * di + c
                        rb4 = rows4.tile([1, 512], F32, name="rb4b", tag="rb4")
                        nc.scalar.copy(rb4[:], psG2[:])
                        nc.scalar.dma_start(cc3_in[r:r + 1, :], rb4[:])
            nc.gpsimd.collective_compute("AllReduce", ALU.add, replica_groups=RG,
                                         ins=[cc3_in[:]], outs=[cc3_out[:]])

            # ---------------- gate math (replicated) ----------------
            # cc3 buf offsets: r_f 0:2048 | z_f 2048:4096 | r_b 4096:6144 |
            # z_b 6144:8192 | in_f 8192:10240 | in_b 10240:12288 |
            # hn_f 12288:14336 | hn_b 14336:16384
            # read back as [32,128] tiles, rows 0:16 = f, 16:32 = b
            # buf rows(512): 0:8 rzx_f | 8:16 rzx_b | 16:24 rzh_f | 24:32 rzh_b
            #                | 32:36 in_f | 36:40 in_b | 40:44 hn_f | 44:48 hn_b
            cc3v = cc3_out[:].rearrange("a b -> (a b)").rearrange("(p t) -> p t", t=128)
            rt = vp.tile([32, 128], F32)
            nc.gpsimd.dma_start(rt[0:16, :], cc3v[0:16, :])
            nc.gpsimd.dma_start(rt[16:32, :], cc3v[32:48, :])
            zt_ = vp.tile([32, 128], F32)
            nc.gpsimd.dma_start(zt_[0:16, :], cc3v[16:32, :])
            nc.gpsimd.dma_start(zt_[16:32, :], cc3v[48:64, :])
            rh = vp.tile([32, 128], F32)
            nc.gpsimd.dma_start(rh[0:16, :], cc3v[64:80, :])
            nc.gpsimd.dma_start(rh[16:32, :], cc3v[96:112, :])
            zh = vp.tile([32, 128], F32)
            nc.gpsimd.dma_start(zh[0:16, :], cc3v[80:96, :])
            nc.gpsimd.dma_start(zh[16:32, :], cc3v[112:128, :])
            it_ = vp.tile([32, 128], F32)
            nc.gpsimd.dma_start(it_[:], cc3v[128:160, :])
            ht_ = vp.tile([32, 128], F32)
            nc.gpsimd.dma_start(ht_[:], cc3v[160:192, :])
            nc.vector.tensor_tensor(rt[:], rt[:], rh[:], op=ALU.add)
            nc.vector.tensor_tensor(zt_[:], zt_[:], zh[:], op=ALU.add)
            # biases, same split (host supplies matching layouts)
            nc.vector.tensor_tensor(rt[:], rt[:], gbr[:], op=ALU.add)
            nc.vector.tensor_tensor(zt_[:], zt_[:], gbz[:], op=ALU.add)
            nc.vector.tensor_tensor(it_[:], it_[:], gbi[:], op=ALU.add)
            nc.vector.tensor_tensor(ht_[:], ht_[:], gbh[:], op=ALU.add)
            sigr = vp.tile([32, 128], F32)
            nc.scalar.activation(sigr[:], rt[:], AF.Sigmoid)
            sigz = vp.tile([32, 128], F32)
            nc.scalar.activation(sigz[:], zt_[:], AF.Sigmoid)
            narg = vp.tile([32, 128], F32)
            nc.vector.tensor_tensor(narg[:], sigr[:], ht_[:], op=ALU.mult)
            nc.vector.tensor_tensor(narg[:], narg[:], it_[:], op=ALU.add)
            ngate = vp.tile([32, 128], F32)
            nc.scalar.activation(ngate[:], narg[:], AF.Tanh)
            dlt = vp.tile([32, 128], F32)
            nc.vector.tensor_tensor(dlt[:], hprev[:], ngate[:], op=ALU.subtract)
            zd = vp.tile([32, 128], F32)
            nc.vector.tensor_tensor(zd[:], sigz[:], dlt[:], op=ALU.mult)
            h_new = vp.tile([32, 128], F32)
            nc.vector.tensor_tensor(h_new[:], ngate[:], zd[:], op=ALU.add)
            nc.gpsimd.dma_start(nh_d[:], h_new[:])
            psGc = psv.tile([128, 32], F32, name="psGc", tag="ptrans")
            nc.tensor.matmul(psGc[:], h_new[:], id32[:, :], is_transpose=True,
                             start=True, stop=True)
            gru_cols = vp.tile([128, 32], F32)
            nc.vector.tensor_copy(gru_cols[:], psGc[:])
            gru_colsw = gru_cols if wdt == F32 else cast_cols(gru_cols, "grucols")

            # ---------------- stage 5: vocab projection ----------------
            ldram = dr.tile([13, 512], F32)
            ses = vp.tile([1, 16], F32)
            junk5 = vp.tile([1, 512], F32)
            with tc.tile_pool(name="rows5", bufs=2) as rows5, \
                 tc.tile_pool(name="s5p", bufs=2, space="PSUM") as s5p:
                obf = rows5.tile([1, VC], F32, name="obf", tag="obf", bufs=1)
                nc.scalar.dma_start(obf[:], obf_d[:])
                for c in range(13):
                    if c in wo_tiles:
                        wo = wo_tiles[c]
                    else:
                        wo = s5w.tile([128, 32, 512], wdt, name="wo", tag="wo")
                        nc.sync.dma_start(wo[:], wout_d[:, c])
                    psO = s5p.tile([1, 512], F32, name="psO", tag="psO")
                    for t in range(32):
                        _mm(nc, psO[:], gru_colsw[:, t:t + 1], wo[:, t, :],
                            t == 0, t == 31, dt_mode)
                    rb5 = rows5.tile([1, 512], F32, name="rb5", tag="rb5")
                    nc.vector.tensor_tensor(rb5[:], psO[:],
                                            obf[0:1, bass.ts(c, 512)], op=ALU.add)
                    nc.scalar.dma_start(ldram[c:c + 1, :], rb5[:])
                    nc.scalar.activation(junk5[:], rb5[:], AF.Exp,
                                         accum_out=ses[0:1, c:c + 1])
            seZ = vp.tile([1, 1], F32)
            nc.vector.reduce_sum(seZ[:], ses[0:1, 0:13], axis=mybir.AxisListType.X)
            nc.scalar.dma_start(cc4_in[:], seZ[:])
            nc.gpsimd.collective_compute("AllReduce", ALU.add, replica_groups=RG,
                                         ins=[cc4_in[:]], outs=[cc4_out[:]])
            # biased-logits readback overlaps the final AllReduce
            logits_sb = vp.tile([13, 512], F32)
            nc.gpsimd.dma_start(logits_sb[:], ldram[:])
            zt = vp.tile([1, 1], F32)
            nc.gpsimd.dma_start(zt[:], cc4_out[:])
            lnz = vp.tile([1, 1], F32)
            nc.scalar.activation(lnz[:], zt[:], AF.Ln)
            nlz = vp.tile([1, 1], F32)
            nc.vector.tensor_scalar(nlz[:], lnz[:], -1.0, None, ALU.mult)
            psB = psv.tile([13, 1], F32, name="psB", tag="psmall")
            nc.tensor.matmul(psB[:], ones113[:], nlz[:], start=True, stop=True)
            nlz13 = vp.tile([13, 1], F32)
            nc.scalar.copy(nlz13[:], psB[:])
            logp_sb = vp.tile([13, 512], F32)
            nc.scalar.activation(logp_sb[:], logits_sb[:], AF.Identity,
                                 bias=nlz13[:, :1])
            nc.gpsimd.dma_start(lp_d[:], logp_sb[:])

    nc.compile()
    return nc


def _tiles(mat, nt, nch):
    """[nt*128, nch*512] -> [128, nch, nt, 512] host blocking."""
    r, c = mat.shape
    assert r == nt * 128 and c == nch * 512, (mat.shape, nt, nch)
    return np.ascontiguousarray(
        mat.reshape(nt, 128, nch, 512).transpose(1, 2, 0, 3))


def _prep_inputs(inputs):
    f32 = np.float32
    input_ids = np.asarray(inputs["input_ids"])
    hidden = np.asarray(inputs["hidden"], f32)
    latent_out = np.asarray(inputs["latent_out"], f32)
    emb = np.asarray(inputs["emb"], f32)
    attn_W = np.asarray(inputs["attn_W"], f32)
    attn_b = np.asarray(inputs["attn_b"], f32)
    comb_W = np.asarray(inputs["comb_W"], f32)
    comb_b = np.asarray(inputs["comb_b"], f32)
    out_W = np.asarray(inputs["out_W"], f32)
    out_b = np.asarray(inputs["out_b"], f32)
    Wih = {"f": np.asarray(inputs["Wih_f"], f32), "b": np.asarray(inputs["Wih_b"], f32)}
    Whh = {"f": np.asarray(inputs["Whh_f"], f32), "b": np.asarray(inputs["Whh_b"], f32)}
    bih = {"f": np.asarray(inputs["bih_f"], f32), "b": np.asarray(inputs["bih_b"], f32)}
    bhh = {"f": np.asarray(inputs["bhh_f"], f32), "b": np.asarray(inputs["bhh_b"], f32)}

    idx = int(np.asarray(input_ids).reshape(-1)[0])
    hflat = hidden.reshape(-1)                      # [4096] = [h_f; h_b]
    aWT = np.ascontiguousarray(attn_W.T)            # [8192, 512]
    cWT = np.ascontiguousarray(comb_W.T)            # [8192, 4096]
    WihT = {d: np.ascontiguousarray(Wih[d].T) for d in "fb"}   # [4096, 6144]
    WhhT = {d: np.ascontiguousarray(Whh[d].T) for d in "fb"}   # [2048, 6144]
    oWTp = np.zeros((H, VP), f32)                   # [4096, 53248]
    oWTp[:, :V] = out_W.T
    obp = np.full((VP,), NEG, f32)
    obp[:V] = out_b
    # gate biases as [32,128] (rows 0:16 fwd, 16:32 bwd)
    bsum = {d: bih[d] + bhh[d] for d in "fb"}
    gbr = np.concatenate([bsum["f"][0:2048], bsum["b"][0:2048]]).reshape(32, 128)
    gbz = np.concatenate([bsum["f"][2048:4096], bsum["b"][2048:4096]]).reshape(32, 128)
    gbi = np.concatenate([bih["f"][4096:6144], bih["b"][4096:6144]]).reshape(32, 128)
    gbh = np.concatenate([bhh["f"][4096:6144], bhh["b"][4096:6144]]).reshape(32, 128)

    if DT_MODE in ("bf16", "mixed"):
        import ml_dtypes
        bfc = lambda a: np.ascontiguousarray(a).astype(ml_dtypes.bfloat16)
    if DT_MODE == "bf16":
        wcast = bfc                     # attn group too
        acast = bfc
        def gcast(a):                   # comb/gru: add nv=1 axis
            return bfc(a)[:, :, None]
    elif DT_MODE == "mixed":
        acast = lambda a: a             # attn stays f32
        wcast = bfc                     # out projection bf16
        def gcast(a):                   # comb/gru: (hi, lo) pair axis
            hi = bfc(a)
            lo = bfc(a - hi.astype(np.float32))
            return np.ascontiguousarray(np.stack([hi, lo], axis=2))
    else:
        wcast = lambda a: a
        acast = lambda a: a
        gcast = lambda a: np.ascontiguousarray(a)[:, :, None]

    eye32 = np.eye(32, dtype=f32)
    in_maps = []
    for k in range(M):
        s = 512 * k
        hs = 256 * k
        im = {
            "embp": np.ascontiguousarray(emb[:, s:s + 512]),
            "idx2": np.array([[idx], [idx]], np.int32),
            "xrows": (4 * k + np.arange(4, dtype=np.int32)).reshape(4, 1),
            "id32": eye32,
            "hkc": np.ascontiguousarray(hflat[s:s + 512].reshape(4, 128).T),
            "hfkc": np.ascontiguousarray(hidden[0, 0, hs:hs + 256].reshape(2, 128).T),
            "hbkc": np.ascontiguousarray(hidden[1, 0, hs:hs + 256].reshape(2, 128).T),
            "hprev": np.ascontiguousarray(hidden.reshape(32, 128)),
            "abcm": attn_b.reshape(4, 128),
            "cbcm": np.ascontiguousarray(comb_b[s:s + 512].reshape(4, 128)),
            "gbr": gbr, "gbz": gbz, "gbi": gbi, "gbh": gbh,
            "obf": np.ascontiguousarray(obp[VC * k:VC * (k + 1)].reshape(1, VC)),
            "a1": acast(_tiles(aWT[s:s + 512], 4, 1)),
            "a2": acast(_tiles(aWT[4096 + s:4096 + s + 512], 4, 1)),
            "lat": acast(_tiles(np.ascontiguousarray(latent_out[:, s:s + 512]), 4, 1)),
            "c1": gcast(_tiles(cWT[s:s + 512], 4, 8)),
            "c2": gcast(_tiles(cWT[4096 + s:4096 + s + 512], 4, 8)),
            "wout": wcast(_tiles(np.ascontiguousarray(oWTp[:, VC * k:VC * (k + 1)]),
                                 32, 13)),
        }
        for d in "fb":
            im[f"wrz{d}"] = gcast(_tiles(WihT[d][s:s + 512, 0:4096], 4, 8))
            im[f"whrz{d}"] = gcast(_tiles(WhhT[d][hs:hs + 256, 0:4096], 2, 8))
            im[f"win{d}"] = gcast(_tiles(WihT[d][s:s + 512, 4096:6144], 4, 4))
            im[f"whn{d}"] = gcast(_tiles(WhhT[d][hs:hs + 256, 4096:6144], 2, 4))
        in_maps.append(im)
    return in_maps


def run(inputs, trace=False):
    if DT_MODE not in _CACHE:
        _CACHE[DT_MODE] = _build(DT_MODE)
    nc = _CACHE[DT_MODE]
    in_maps = _prep_inputs(inputs)
    res = run_bass_kernel_spmd(nc, in_maps, core_ids=list(range(M)), trace=trace)
    logp = np.concatenate([res.results[c]["lp"].reshape(-1) for c in range(M)])
    logp = logp[:V].reshape(1, V)
    new_hidden = res.results[0]["nh"].reshape(2, 1, Hh)
    attn_weights = res.results[0]["aw"].reshape(1, L)
    return (logp, new_hidden, attn_weights), res


def kernel(**inputs):
    out, _ = run(inputs, trace=False)
    return out


# revision 24
# speedup vs baseline: 1.1168x; 1.0746x over previous
"""AttnDecoderRNN step on 8 TRN2 NeuronCores, tensor-parallel.

Strategy (contraction sharding + AllReduce between stages):
  stage1 attn logits : attn_W.T row-sharded (512 emb rows + 512 hid rows
                       per core), partial [512] -> AR -> +bias -> softmax
  stage2 attn_applied: latent_out column-sharded; each core computes its
                       own 512-wide slice (no collective)
  stage3 comb        : comb_W.T row-sharded (512 emb + 512 attn rows per
                       core), partial [4096] -> AR -> slice readback ->
                       +bias -> relu -> x slice
  stage4 GRU gates   : Wih/Whh contraction-sharded (512 x-rows, 256 h-rows
                       per core); r/z gates pre-summed (gi+gh) in PSUM, the
                       n gate halves kept split; partial [16384] -> AR ->
                       +bias -> gate math -> h_new
  stage5 vocab proj  : out_W output-sharded (6656 rows per core, padded to
                       53248); +bias, local sumexp -> AR(scalar) -> logp
Embedding lookup: emb column-sharded; one indirect-DMA row gather per core.
All weights are pre-transposed/tiled on the host so every DMA is contiguous
and every matmul operand sits at partition base 0.  Matvec form:
psum[1,N] += lhsT(x column tile [128,1]).T @ rhs(W.T tile [128,N]).
Biases are applied once, post-AllReduce, as elementwise adds.
"""
import os
import sys

sys.path.insert(0, "/opt/trn_rl_repo")

import numpy as np

from concourse import bass, bacc, tile, mybir
from concourse.bass_utils import run_bass_kernel_spmd

V, H, L, Hh = 50257, 4096, 512, 2048
M = 8               # cores
VC = 6656           # per-core padded vocab rows (13 * 512)
VP = VC * M         # 53248
NEG = -1.0e30

F32 = mybir.dt.float32
F32R = mybir.dt.float32r
BF16 = mybir.dt.bfloat16
I32 = mybir.dt.int32

# 'f32' (exact, PE 4cyc/row) or 'bf16' (half DMA bytes, PE 1cyc/row)
DT_MODE = os.environ.get("BASS_KERNEL_DT", "f32")

AF = mybir.ActivationFunctionType
ALU = mybir.AluOpType
RG = [list(range(M))]

_CACHE = {}


def _mm(nc, out, lhsT, rhs, start, stop, dt_mode):
    if dt_mode == "f32r":
        lhsT = lhsT.bitcast(F32R)
        rhs = rhs.bitcast(F32R)
    nc.tensor.matmul(out, lhsT, rhs, start=start, stop=stop)


def _build(dt_mode):
    wdt = {"f32": F32, "f32r": F32, "bf16": BF16, "mixed": BF16}[dt_mode]
    wdt_at = F32 if dt_mode in ("f32", "f32r", "mixed") else BF16
    pair = dt_mode == "mixed"
    nv = 2 if pair else 1
    nc = bacc.Bacc("TRN2", target_bir_lowering=False, debug=False,
                   enable_asserts=True, num_devices=M)

    def din(name, shape, dt=F32):
        return nc.dram_tensor(name, shape, dt, kind="ExternalInput").ap()

    def dout(name, shape, dt=F32):
        return nc.dram_tensor(name, shape, dt, kind="ExternalOutput").ap()

    embp = din("embp", [V, 512])
    idx2 = din("idx2", [2, 1], I32)
    xrows = din("xrows", [4, 1], I32)
    id32_d = din("id32", [32, 32])
    hkc_d = din("hkc", [128, 4])
    hfkc_d = din("hfkc", [128, 2])
    hbkc_d = din("hbkc", [128, 2])
    hprev_d = din("hprev", [32, 128])
    abcm_d = din("abcm", [4, 128])
    cbcm_d = din("cbcm", [4, 128])
    gbr_d = din("gbr", [32, 128])
    gbz_d = din("gbz", [32, 128])
    gbi_d = din("gbi", [32, 128])
    gbh_d = din("gbh", [32, 128])
    obf_d = din("obf", [1, VC])
    a1_d = din("a1", [128, 4, 512], wdt_at)
    a2_d = din("a2", [128, 4, 512], wdt_at)
    lat_d = din("lat", [128, 4, 512], wdt_at)
    c1_d = din("c1", [128, 8, nv, 4, 512], wdt)
    c2_d = din("c2", [128, 8, nv, 4, 512], wdt)
    wrzf_d = din("wrzf", [128, 8, nv, 4, 512], wdt)
    wrzb_d = din("wrzb", [128, 8, nv, 4, 512], wdt)
    whrzf_d = din("whrzf", [128, 8, nv, 2, 512], wdt)
    whrzb_d = din("whrzb", [128, 8, nv, 2, 512], wdt)
    winf_d = din("winf", [128, 4, nv, 4, 512], wdt)
    winb_d = din("winb", [128, 4, nv, 4, 512], wdt)
    whnf_d = din("whnf", [128, 4, nv, 2, 512], wdt)
    whnb_d = din("whnb", [128, 4, nv, 2, 512], wdt)
    wout_d = din("wout", [128, 13, 32, 512], wdt)

    lp_d = dout("lp", [13, 512])
    nh_d = dout("nh", [32, 128])
    aw_d = dout("aw", [4, 128])

    with tile.TileContext(nc) as tc:
        PF = (3 if pair else 4) if wdt == BF16 else 1
        with tc.tile_pool(name="const", bufs=1) as cp, \
             tc.tile_pool(name="vec", bufs=1) as vp, \
             tc.tile_pool(name="psv", bufs=2, space="PSUM") as psv, \
             tc.tile_pool(name="s5w", bufs=max(PF, 2)) as s5w, \
             tc.tile_pool(name="dram", bufs=1, space="DRAM") as dr:

            id32 = cp.tile([32, 32], F32)
            nc.sync.dma_start(id32[:], id32_d[:])
            ones4 = cp.tile([4, 1], F32)
            nc.vector.memset(ones4[:], 1.0)
            ones13 = cp.tile([13, 1], F32)
            nc.vector.memset(ones13[:], 1.0)
            ones113 = cp.tile([1, 13], F32)
            nc.vector.memset(ones113[:], 1.0)
            ones14 = cp.tile([1, 4], F32)
            nc.vector.memset(ones14[:], 1.0)

            idx2_s = cp.tile([2, 1], I32)
            nc.sync.dma_start(idx2_s[:], idx2[:])
            xrows_s = cp.tile([4, 1], I32)
            nc.sync.dma_start(xrows_s[:], xrows[:])
            hkc = cp.tile([128, 4], F32)
            nc.sync.dma_start(hkc[:], hkc_d[:])
            hfkc = cp.tile([128, 2], F32)
            nc.sync.dma_start(hfkc[:], hfkc_d[:])
            hbkc = cp.tile([128, 2], F32)
            nc.sync.dma_start(hbkc[:], hbkc_d[:])
            hprev = cp.tile([32, 128], F32)
            nc.sync.dma_start(hprev[:], hprev_d[:])
            abcm = cp.tile([4, 128], F32)
            nc.sync.dma_start(abcm[:], abcm_d[:])
            cbcm = cp.tile([4, 128], F32)
            nc.sync.dma_start(cbcm[:], cbcm_d[:])
            gbr = cp.tile([32, 128], F32)
            nc.sync.dma_start(gbr[:], gbr_d[:])
            gbz = cp.tile([32, 128], F32)
            nc.sync.dma_start(gbz[:], gbz_d[:])
            gbi = cp.tile([32, 128], F32)
            nc.sync.dma_start(gbi[:], gbi_d[:])
            gbh = cp.tile([32, 128], F32)
            nc.sync.dma_start(gbh[:], gbh_d[:])

            s1w_cm = tc.tile_pool(name="s1w", bufs=1)
            s1w = s1w_cm.__enter__()
            a1 = s1w.tile([128, 4, 512], wdt_at)
            nc.sync.dma_start(a1[:], a1_d[:])
            a2 = s1w.tile([128, 4, 512], wdt_at)
            nc.sync.dma_start(a2[:], a2_d[:])
            lat = s1w.tile([128, 4, 512], wdt_at)
            nc.sync.dma_start(lat[:], lat_d[:])

            # collective bounce buffers
            cc1_in = dr.tile([1, 512], F32)
            cc1_out = dr.tile([1, 512], F32, addr_space="Shared")
            cc2_in = dr.tile([8, 512], F32)
            cc2_out = dr.tile([8, 512], F32, addr_space="Shared")
            cc3_in = dr.tile([48, 512], F32)
            cc3_out = dr.tile([48, 512], F32, addr_space="Shared")
            cc4_in = dr.tile([1, 1], F32)
            cc4_out = dr.tile([1, 1], F32, addr_space="Shared")

            def row_to_cols(row, n, name):
                """[1, 128*n] SBUF row -> [128, n] col tiles via PE transposes."""
                pc = psv.tile([128, n], F32, name=f"{name}_ps", tag="ptrans")
                for t in range(n):
                    nc.tensor.matmul(pc[:, t:t + 1], row[0:1, bass.ts(t, 128)],
                                     id32[0:1, 0:1], is_transpose=True,
                                     start=(t == 0), stop=(t == n - 1))
                out = vp.tile([128, n], F32, name=f"{name}_sb", tag=f"{name}_sb")
                nc.vector.tensor_copy(out[:], pc[:])
                return out

            def cast_cols(cols, name):
                if wdt != BF16:
                    return cols
                cb = vp.tile(list(cols.shape), BF16, name=f"{name}_bf",
                             tag=f"{name}_bf")
                nc.vector.tensor_copy(cb[:], cols[:])
                return cb

            def split_cols(cols, name):
                """f32 cols -> (hi, lo) bf16 pair; or single bf16/f32 view."""
                if not pair:
                    if wdt == BF16:
                        cb = vp.tile(list(cols.shape), BF16, name=f"{name}_h",
                                     tag=f"{name}_h")
                        nc.vector.tensor_copy(cb[:], cols[:])
                        return (cb,)
                    return (cols,)
                hi = vp.tile(list(cols.shape), BF16, name=f"{name}_h",
                             tag=f"{name}_h")
                nc.vector.tensor_copy(hi[:], cols[:])
                hif = vp.tile(list(cols.shape), F32, name=f"{name}_hf",
                              tag=f"{name}_hf")
                nc.vector.tensor_copy(hif[:], hi[:])
                dif = vp.tile(list(cols.shape), F32, name=f"{name}_d",
                              tag=f"{name}_d")
                nc.vector.tensor_tensor(dif[:], cols[:], hif[:], op=ALU.subtract)
                lo = vp.tile(list(cols.shape), BF16, name=f"{name}_l",
                             tag=f"{name}_l")
                nc.vector.tensor_copy(lo[:], dif[:])
                return (hi, lo)

            def mm_group(ps, lhs_sets, first, last):
                """Accumulate sum_j lhs_sets[j] @ tiles_j into ps.
                lhs_sets: list of (cols_tuple, wtile_ap, nt); wtile_ap indexed
                [v, t] in pair mode else [t]."""
                seq = []
                for cols, wtile, nt in lhs_sets:
                    for t in range(nt):
                        seq.append((cols[0][:, t:t + 1], wtile[:, 0, t, :]))
                        if pair:
                            seq.append((cols[1][:, t:t + 1], wtile[:, 0, t, :]))
                            seq.append((cols[0][:, t:t + 1], wtile[:, 1, t, :]))
                for i, (l, r) in enumerate(seq):
                    nc.tensor.matmul(ps, l, r,
                                     start=(first and i == 0),
                                     stop=(last and i == len(seq) - 1))

            # ---------------- embedding gather ----------------
            ge = vp.tile([2, 512], F32)
            nc.gpsimd.indirect_dma_start(
                out=ge[:], out_offset=None, in_=embp[:],
                in_offset=bass.IndirectOffsetOnAxis(ap=idx2_s[:, :1], axis=0))
            e_cols = row_to_cols(ge[0:1, :], 4, "ecols")
            e_at = e_cols if wdt_at == F32 else cast_cols(e_cols, "ecat")
            hkc_at = hkc if wdt_at == F32 else cast_cols(hkc, "hkcat")
            e_cg = split_cols(e_cols, "ecg")
            hf_cg = split_cols(hfkc, "hfcg")
            hb_cg = split_cols(hbkc, "hbcg")

            # ---------------- stage 1: attn logits partial ----------------
            with tc.tile_pool(name="s1p", bufs=1, space="PSUM") as s1p:
                psL = s1p.tile([1, 512], F32)
                for t in range(4):
                    _mm(nc, psL[:], e_at[:, t:t + 1], a1[:, t, :],
                        t == 0, False, dt_mode)
                for t in range(4):
                    _mm(nc, psL[:], hkc_at[:, t:t + 1], a2[:, t, :],
                        False, t == 3, dt_mode)
                sL = vp.tile([1, 512], F32)
                nc.scalar.copy(sL[:], psL[:])
                nc.scalar.dma_start(cc1_in[:], sL[:])

            nc.gpsimd.collective_compute("AllReduce", ALU.add, replica_groups=RG,
                                         ins=[cc1_in[:]], outs=[cc1_out[:]])

            # ------- GRU h-side partials (independent of x; run early) -------
            with tc.tile_pool(name="s0w", bufs=3) as s0w, \
                 tc.tile_pool(name="rows0", bufs=2) as rows0, \
                 tc.tile_pool(name="s0p", bufs=2, space="PSUM") as s0p:
                for di, (whrz_d, hcw) in enumerate(((whrzf_d, hf_cg), (whrzb_d, hb_cg))):
                    for c in range(8):
                        wh = s0w.tile([128, nv, 2, 512], wdt, name="wh", tag="wh")
                        nc.sync.dma_start(wh[:], whrz_d[:, c])
                        psH = s0p.tile([1, 512], F32, name="psH", tag="psH")
                        mm_group(psH[:], [(hcw, wh, 2)], True, True)
                        rb0 = rows0.tile([1, 512], F32, name="rb0", tag="rb0")
                        nc.scalar.copy(rb0[:], psH[:])
                        nc.scalar.dma_start(cc3_in[16 + 8 * di + c:17 + 8 * di + c, :], rb0[:])
                for di, (whn_d, hcw) in enumerate(((whnf_d, hf_cg), (whnb_d, hb_cg))):
                    for c in range(4):
                        wh2 = s0w.tile([128, nv, 2, 512], wdt, name="wh2", tag="wh")
                        nc.sync.dma_start(wh2[:], whn_d[:, c])
                        psH2 = s0p.tile([1, 512], F32, name="psH2", tag="psH")
                        mm_group(psH2[:], [(hcw, wh2, 2)], True, True)
                        rb0 = rows0.tile([1, 512], F32, name="rb0b", tag="rb0")
                        nc.scalar.copy(rb0[:], psH2[:])
                        nc.scalar.dma_start(cc3_in[40 + 4 * di + c:41 + 4 * di + c, :], rb0[:])

            # ---------------- softmax (replicated) ----------------
            lg = vp.tile([4, 128], F32)
            nc.gpsimd.dma_start(lg[:], cc1_out[:].rearrange("a (p t) -> (a p) t", t=128))
            nc.vector.tensor_tensor(lg[:], lg[:], abcm[:], op=ALU.add)
            exw = vp.tile([4, 128], F32)
            exs = vp.tile([4, 1], F32)
            nc.scalar.activation(exw[:], lg[:], AF.Exp, accum_out=exs[:])
            psZ = psv.tile([1, 1], F32, name="psZ", tag="psmall")
            nc.tensor.matmul(psZ[:], exs[:], ones4[:], start=True, stop=True)
            sZ = vp.tile([1, 1], F32)
            nc.scalar.copy(sZ[:], psZ[:])
            rZ = vp.tile([1, 1], F32)
            nc.vector.reciprocal(rZ[:], sZ[:])
            psR = psv.tile([4, 1], F32, name="psR", tag="psmall")
            nc.tensor.matmul(psR[:], ones14[:], rZ[:], start=True, stop=True)
            rZ4 = vp.tile([4, 1], F32)
            nc.scalar.copy(rZ4[:], psR[:])
            w_cm = vp.tile([4, 128], F32)
            nc.vector.tensor_scalar(w_cm[:], exw[:], rZ4[:, :1], None, ALU.mult)
            nc.gpsimd.dma_start(aw_d[:], w_cm[:])
            psWc = psv.tile([128, 4], F32, name="psWc", tag="ptrans")
            nc.tensor.matmul(psWc[:], w_cm[:], id32[0:4, 0:4], is_transpose=True,
                             start=True, stop=True)
            w_cols = vp.tile([128, 4], F32)
            nc.vector.tensor_copy(w_cols[:], psWc[:])
            w_at = w_cols if wdt_at == F32 else cast_cols(w_cols, "wat")

            # ---------------- stage 2: attn_applied slice ----------------
            with tc.tile_pool(name="s2p", bufs=1, space="PSUM") as s2p:
                psA = s2p.tile([1, 512], F32)
                for t in range(4):
                    _mm(nc, psA[:], w_at[:, t:t + 1], lat[:, t, :],
                        t == 0, t == 3, dt_mode)
                sA = vp.tile([1, 512], F32)
                nc.scalar.copy(sA[:], psA[:])
            a_cols = row_to_cols(sA, 4, "acols")
            a_cg = split_cols(a_cols, "acg")
            s1w_cm.__exit__(None, None, None)

            # ---------------- stage 3: comb partial ----------------
            with tc.tile_pool(name="s3w", bufs=(6 if wdt == BF16 and nv == 1 else 3)) as s3w, \
                 tc.tile_pool(name="rows3", bufs=2) as rows3, \
                 tc.tile_pool(name="s3p", bufs=2, space="PSUM") as s3p:
                for c in range(8):
                    c1t = s3w.tile([128, nv, 4, 512], wdt, name="c1t", tag="ct")
                    nc.sync.dma_start(c1t[:], c1_d[:, c])
                    c2t = s3w.tile([128, nv, 4, 512], wdt, name="c2t", tag="ct")
                    nc.sync.dma_start(c2t[:], c2_d[:, c])
                    psC = s3p.tile([1, 512], F32, name="psC", tag="psC")
                    mm_group(psC[:], [(e_cg, c1t, 4), (a_cg, c2t, 4)], True, True)
                    rb3 = rows3.tile([1, 512], F32, name="rb3", tag="rb3")
                    nc.scalar.copy(rb3[:], psC[:])
                    nc.scalar.dma_start(cc2_in[c:c + 1, :], rb3[:])

            # prefetch first wout chunks while the attn/comb ARs run
            wo_tiles = {}
            for c in range(PF):
                wo = s5w.tile([128, 32, 512], wdt, name="wo", tag="wo")
                (nc.sync if c % 2 == 0 else nc.scalar).dma_start(wo[:], wout_d[:, c])
                wo_tiles[c] = wo

            nc.gpsimd.collective_compute("AllReduce", ALU.add, replica_groups=RG,
                                         ins=[cc2_in[:]], outs=[cc2_out[:]])

            # core-local x slice readback (rows 4k..4k+3 of [32,128] view)
            xr4 = vp.tile([4, 128], F32)
            nc.gpsimd.indirect_dma_start(
                out=xr4[:], out_offset=None,
                in_=cc2_out[:].rearrange("a (p t) -> (a p) t", t=128),
                in_offset=bass.IndirectOffsetOnAxis(ap=xrows_s[:, :1], axis=0))
            nc.vector.tensor_tensor(xr4[:], xr4[:], cbcm[:], op=ALU.add)
            xrelu = vp.tile([4, 128], F32)
            nc.scalar.activation(xrelu[:], xr4[:], AF.Relu)
            psXc = psv.tile([128, 4], F32, name="psXc", tag="ptrans")
            nc.tensor.matmul(psXc[:], xrelu[:], id32[0:4, 0:4], is_transpose=True,
                             start=True, stop=True)
            xk_cols = vp.tile([128, 4], F32)
            nc.vector.tensor_copy(xk_cols[:], psXc[:])
            xk_cg = split_cols(xk_cols, "xkcg")

            # ---------------- stage 4: GRU gate partials ----------------
            # gates_sb rows (512 each): 0:8 rz_f | 8:16 rz_b | 16:20 in_f |
            # 20:24 in_b | 24:28 hn_f | 28:32 hn_b
            with tc.tile_pool(name="s4w", bufs=(6 if wdt == BF16 and nv == 1 else 3)) as s4w, \
                 tc.tile_pool(name="rows4", bufs=2) as rows4, \
                 tc.tile_pool(name="s4p", bufs=2, space="PSUM") as s4p:
                for di, wrz_d in enumerate((wrzf_d, wrzb_d)):
                    for c in range(8):
                        wt = s4w.tile([128, nv, 4, 512], wdt, name="wrzt", tag="gt")
                        nc.sync.dma_start(wt[:], wrz_d[:, c])
                        psG = s4p.tile([1, 512], F32, name="psG", tag="psG")
                        mm_group(psG[:], [(xk_cg, wt, 4)], True, True)
                        r = 8 * di + c
                        rb4 = rows4.tile([1, 512], F32, name="rb4", tag="rb4")
                        nc.scalar.copy(rb4[:], psG[:])
                        nc.scalar.dma_start(cc3_in[r:r + 1, :], rb4[:])
                for di, win_d in enumerate((winf_d, winb_d)):
                    for c in range(4):
                        wt2 = s4w.tile([128, nv, 4, 512], wdt, name="wint", tag="gt")
                        nc.sync.dma_start(wt2[:], win_d[:, c])
                        psG2 = s4p.tile([1, 512], F32, name="psG2", tag="psG")
                        mm_group(psG2[:], [(xk_cg, wt2, 4)], True, True)
                        r = 32 + 4 * di + c
                        rb4 = rows4.tile([1, 512], F32, name="rb4b", tag="rb4")
                        nc.scalar.copy(rb4[:], psG2[:])
                        nc.scalar.dma_start(cc3_in[r:r + 1, :], rb4[:])
            nc.gpsimd.collective_compute("AllReduce", ALU.add, replica_groups=RG,
                                         ins=[cc3_in[:]], outs=[cc3_out[:]])

            # ---------------- gate math (replicated) ----------------
            # cc3 buf offsets: r_f 0:2048 | z_f 2048:4096 | r_b 4096:6144 |
            # z_b 6144:8192 | in_f 8192:10240 | in_b 10240:12288 |
            # hn_f 12288:14336 | hn_b 14336:16384
            # read back as [32,128] tiles, rows 0:16 = f, 16:32 = b
            # buf rows(512): 0:8 rzx_f | 8:16 rzx_b | 16:24 rzh_f | 24:32 rzh_b
            #                | 32:36 in_f | 36:40 in_b | 40:44 hn_f | 44:48 hn_b
            cc3v = cc3_out[:].rearrange("a b -> (a b)").rearrange("(p t) -> p t", t=128)
            rt = vp.tile([32, 128], F32)
            nc.gpsimd.dma_start(rt[0:16, :], cc3v[0:16, :])
            nc.gpsimd.dma_start(rt[16:32, :], cc3v[32:48, :])
            zt_ = vp.tile([32, 128], F32)
            nc.gpsimd.dma_start(zt_[0:16, :], cc3v[16:32, :])
            nc.gpsimd.dma_start(zt_[16:32, :], cc3v[48:64, :])
            rh = vp.tile([32, 128], F32)
            nc.gpsimd.dma_start(rh[0:16, :], cc3v[64:80, :])
            nc.gpsimd.dma_start(rh[16:32, :], cc3v[96:112, :])
            zh = vp.tile([32, 128], F32)
            nc.gpsimd.dma_start(zh[0:16, :], cc3v[80:96, :])
            nc.gpsimd.dma_start(zh[16:32, :], cc3v[112:128, :])
            it_ = vp.tile([32, 128], F32)
            nc.gpsimd.dma_start(it_[:], cc3v[128:160, :])
            ht_ = vp.tile([32, 128], F32)
            nc.gpsimd.dma_start(ht_[:], cc3v[160:192, :])
            nc.vector.tensor_tensor(rt[:], rt[:], rh[:], op=ALU.add)
            nc.vector.tensor_tensor(zt_[:], zt_[:], zh[:], op=ALU.add)
            # biases, same split (host supplies matching layouts)
            nc.vector.tensor_tensor(rt[:], rt[:], gbr[:], op=ALU.add)
            nc.vector.tensor_tensor(zt_[:], zt_[:], gbz[:], op=ALU.add)
            nc.vector.tensor_tensor(it_[:], it_[:], gbi[:], op=ALU.add)
            nc.vector.tensor_tensor(ht_[:], ht_[:], gbh[:], op=ALU.add)
            sigr = vp.tile([32, 128], F32)
            nc.scalar.activation(sigr[:], rt[:], AF.Sigmoid)
            sigz = vp.tile([32, 128], F32)
            nc.scalar.activation(sigz[:], zt_[:], AF.Sigmoid)
            narg = vp.tile([32, 128], F32)
            nc.vector.tensor_tensor(narg[:], sigr[:], ht_[:], op=ALU.mult)
            nc.vector.tensor_tensor(narg[:], narg[:], it_[:], op=ALU.add)
            ngate = vp.tile([32, 128], F32)
            nc.scalar.activation(ngate[:], narg[:], AF.Tanh)
            dlt = vp.tile([32, 128], F32)
            nc.vector.tensor_tensor(dlt[:], hprev[:], ngate[:], op=ALU.subtract)
            zd = vp.tile([32, 128], F32)
            nc.vector.tensor_tensor(zd[:], sigz[:], dlt[:], op=ALU.mult)
            h_new = vp.tile([32, 128], F32)
            nc.vector.tensor_tensor(h_new[:], ngate[:], zd[:], op=ALU.add)
            nc.gpsimd.dma_start(nh_d[:], h_new[:])
            psGc = psv.tile([128, 32], F32, name="psGc", tag="ptrans")
            nc.tensor.matmul(psGc[:], h_new[:], id32[:, :], is_transpose=True,
                             start=True, stop=True)
            gru_cols = vp.tile([128, 32], F32)
            nc.vector.tensor_copy(gru_cols[:], psGc[:])
            gru_colsw = gru_cols if wdt == F32 else cast_cols(gru_cols, "grucols")

            # ---------------- stage 5: vocab projection ----------------
            ldram = dr.tile([13, 512], F32)
            ses = vp.tile([1, 16], F32)
            junk5 = vp.tile([1, 512], F32)
            with tc.tile_pool(name="rows5", bufs=2) as rows5, \
                 tc.tile_pool(name="s5p", bufs=2, space="PSUM") as s5p:
                obf = rows5.tile([1, VC], F32, name="obf", tag="obf", bufs=1)
                nc.scalar.dma_start(obf[:], obf_d[:])
                for c in range(13):
                    if c in wo_tiles:
                        wo = wo_tiles[c]
                    else:
                        wo = s5w.tile([128, 32, 512], wdt, name="wo", tag="wo")
                        (nc.sync if c % 2 == 0 else nc.scalar).dma_start(wo[:], wout_d[:, c])
                    psO = s5p.tile([1, 512], F32, name="psO", tag="psO")
                    for t in range(32):
                        _mm(nc, psO[:], gru_colsw[:, t:t + 1], wo[:, t, :],
                            t == 0, t == 31, dt_mode)
                    rb5 = rows5.tile([1, 512], F32, name="rb5", tag="rb5")
                    nc.vector.tensor_tensor(rb5[:], psO[:],
                                            obf[0:1, bass.ts(c, 512)], op=ALU.add)
                    nc.scalar.dma_start(ldram[c:c + 1, :], rb5[:])
                    nc.scalar.activation(junk5[:], rb5[:], AF.Exp,
                                         accum_out=ses[0:1, c:c + 1])
            seZ = vp.tile([1, 1], F32)
            nc.vector.reduce_sum(seZ[:], ses[0:1, 0:13], axis=mybir.AxisListType.X)
            nc.scalar.dma_start(cc4_in[:], seZ[:])
            nc.gpsimd.collective_compute("AllReduce", ALU.add, replica_groups=RG,
                                         ins=[cc4_in[:]], outs=[cc4_out[:]])
            # biased-logits readback overlaps the final AllReduce
            logits_sb = vp.tile([13, 512], F32)
            nc.gpsimd.dma_start(logits_sb[:], ldram[:])
            zt = vp.tile([1, 1], F32)
            nc.gpsimd.dma_start(zt[:], cc4_out[:])
            lnz = vp.tile([1, 1], F32)
            nc.scalar.activation(lnz[:], zt[:], AF.Ln)
            nlz = vp.tile([1, 1], F32)
            nc.vector.tensor_scalar(nlz[:], lnz[:], -1.0, None, ALU.mult)
            psB = psv.tile([13, 1], F32, name="psB", tag="psmall")
            nc.tensor.matmul(psB[:], ones113[:], nlz[:], start=True, stop=True)
            nlz13 = vp.tile([13, 1], F32)
            nc.scalar.copy(nlz13[:], psB[:])
            logp_sb = vp.tile([13, 512], F32)
            nc.scalar.activation(logp_sb[:], logits_sb[:], AF.Identity,
                                 bias=nlz13[:, :1])
            nc.gpsimd.dma_start(lp_d[:], logp_sb[:])

    nc.compile()
    return nc


def _tiles(mat, nt, nch):
    """[nt*128, nch*512] -> [128, nch, nt, 512] host blocking."""
    r, c = mat.shape
    assert r == nt * 128 and c == nch * 512, (mat.shape, nt, nch)
    return np.ascontiguousarray(
        mat.reshape(nt, 128, nch, 512).transpose(1, 2, 0, 3))


def _prep_inputs(inputs):
    f32 = np.float32
    input_ids = np.asarray(inputs["input_ids"])
    hidden = np.asarray(inputs["hidden"], f32)
    latent_out = np.asarray(inputs["latent_out"], f32)
    emb = np.asarray(inputs["emb"], f32)
    attn_W = np.asarray(inputs["attn_W"], f32)
    attn_b = np.asarray(inputs["attn_b"], f32)
    comb_W = np.asarray(inputs["comb_W"], f32)
    comb_b = np.asarray(inputs["comb_b"], f32)
    out_W = np.asarray(inputs["out_W"], f32)
    out_b = np.asarray(inputs["out_b"], f32)
    Wih = {"f": np.asarray(inputs["Wih_f"], f32), "b": np.asarray(inputs["Wih_b"], f32)}
    Whh = {"f": np.asarray(inputs["Whh_f"], f32), "b": np.asarray(inputs["Whh_b"], f32)}
    bih = {"f": np.asarray(inputs["bih_f"], f32), "b": np.asarray(inputs["bih_b"], f32)}
    bhh = {"f": np.asarray(inputs["bhh_f"], f32), "b": np.asarray(inputs["bhh_b"], f32)}

    idx = int(np.asarray(input_ids).reshape(-1)[0])
    hflat = hidden.reshape(-1)                      # [4096] = [h_f; h_b]
    aWT = np.ascontiguousarray(attn_W.T)            # [8192, 512]
    cWT = np.ascontiguousarray(comb_W.T)            # [8192, 4096]
    WihT = {d: np.ascontiguousarray(Wih[d].T) for d in "fb"}   # [4096, 6144]
    WhhT = {d: np.ascontiguousarray(Whh[d].T) for d in "fb"}   # [2048, 6144]
    oWTp = np.zeros((H, VP), f32)                   # [4096, 53248]
    oWTp[:, :V] = out_W.T
    obp = np.full((VP,), NEG, f32)
    obp[:V] = out_b
    # gate biases as [32,128] (rows 0:16 fwd, 16:32 bwd)
    bsum = {d: bih[d] + bhh[d] for d in "fb"}
    gbr = np.concatenate([bsum["f"][0:2048], bsum["b"][0:2048]]).reshape(32, 128)
    gbz = np.concatenate([bsum["f"][2048:4096], bsum["b"][2048:4096]]).reshape(32, 128)
    gbi = np.concatenate([bih["f"][4096:6144], bih["b"][4096:6144]]).reshape(32, 128)
    gbh = np.concatenate([bhh["f"][4096:6144], bhh["b"][4096:6144]]).reshape(32, 128)

    if DT_MODE in ("bf16", "mixed"):
        import ml_dtypes
        bfc = lambda a: np.ascontiguousarray(a).astype(ml_dtypes.bfloat16)
    if DT_MODE == "bf16":
        wcast = bfc                     # attn group too
        acast = bfc
        def gcast(a):                   # comb/gru: add nv=1 axis
            return bfc(a)[:, :, None]
    elif DT_MODE == "mixed":
        acast = lambda a: a             # attn stays f32
        wcast = bfc                     # out projection bf16
        def gcast(a):                   # comb/gru: (hi, lo) pair axis
            hi = bfc(a)
            lo = bfc(a - hi.astype(np.float32))
            return np.ascontiguousarray(np.stack([hi, lo], axis=2))
    else:
        wcast = lambda a: a
        acast = lambda a: a
        gcast = lambda a: np.ascontiguousarray(a)[:, :, None]

    eye32 = np.eye(32, dtype=f32)
    in_maps = []
    for k in range(M):
        s = 512 * k
        hs = 256 * k
        im = {
            "embp": np.ascontiguousarray(emb[:, s:s + 512]),
            "idx2": np.array([[idx], [idx]], np.int32),
            "xrows": (4 * k + np.arange(4, dtype=np.int32)).reshape(4, 1),
            "id32": eye32,
            "hkc": np.ascontiguousarray(hflat[s:s + 512].reshape(4, 128).T),
            "hfkc": np.ascontiguousarray(hidden[0, 0, hs:hs + 256].reshape(2, 128).T),
            "hbkc": np.ascontiguousarray(hidden[1, 0, hs:hs + 256].reshape(2, 128).T),
            "hprev": np.ascontiguousarray(hidden.reshape(32, 128)),
            "abcm": attn_b.reshape(4, 128),
            "cbcm": np.ascontiguousarray(comb_b[s:s + 512].reshape(4, 128)),
            "gbr": gbr, "gbz": gbz, "gbi": gbi, "gbh": gbh,
            "obf": np.ascontiguousarray(obp[VC * k:VC * (k + 1)].reshape(1, VC)),
            "a1": acast(_tiles(aWT[s:s + 512], 4, 1)),
            "a2": acast(_tiles(aWT[4096 + s:4096 + s + 512], 4, 1)),
            "lat": acast(_tiles(np.ascontiguousarray(latent_out[:, s:s + 512]), 4, 1)),
            "c1": gcast(_tiles(cWT[s:s + 512], 4, 8)),
            "c2": gcast(_tiles(cWT[4096 + s:4096 + s + 512], 4, 8)),
            "wout": wcast(_tiles(np.ascontiguousarray(oWTp[:, VC * k:VC * (k + 1)]),
                                 32, 13)),
        }
        for d in "fb":
            im[f"wrz{d}"] = gcast(_tiles(WihT[d][s:s + 512, 0:4096], 4, 8))
            im[f"whrz{d}"] = gcast(_tiles(WhhT[d][hs:hs + 256, 0:4096], 2, 8))
            im[f"win{d}"] = gcast(_tiles(WihT[d][s:s + 512, 4096:6144], 4, 4))
            im[f"whn{d}"] = gcast(_tiles(WhhT[d][hs:hs + 256, 4096:6144], 2, 4))
        in_maps.append(im)
    return in_maps


def run(inputs, trace=False):
    if DT_MODE not in _CACHE:
        _CACHE[DT_MODE] = _build(DT_MODE)
    nc = _CACHE[DT_MODE]
    in_maps = _prep_inputs(inputs)
    res = run_bass_kernel_spmd(nc, in_maps, core_ids=list(range(M)), trace=trace)
    logp = np.concatenate([res.results[c]["lp"].reshape(-1) for c in range(M)])
    logp = logp[:V].reshape(1, V)
    new_hidden = res.results[0]["nh"].reshape(2, 1, Hh)
    attn_weights = res.results[0]["aw"].reshape(1, L)
    return (logp, new_hidden, attn_weights), res


def kernel(**inputs):
    out, _ = run(inputs, trace=False)
    return out
